# revision 1
# baseline (speedup 1.0000x reference)
"""Trainium2 Bass kernel for nn_DmTranslateTrain (seq2seq translate train step).

Strategy (8 NeuronCores, SPMD):
  - Data-parallel over batch: core k owns batches [4k, 4k+4). Each core runs the
    full encoder LSTM scan + decoder (LSTM + Luong attention) for its 4 batches.
  - The attention output layer (Wa) is folded on the host into the decoder
    recurrence (Whcomb = 0.5*(Wh_d + Wa_h @ Wxd_a)); the context contribution
    ctx @ (Wa_c @ Wxd_a) is rewritten as align @ (mem @ Wca) -- context lives in
    the 64-dim span of the memory rows, so mem @ Wca is precomputed once after
    the encoder and the per-step matmul contracts over s=64 instead of u=1024.
  - Output projection is tensor-parallel over the vocabulary: one AllGather of
    attention activations, then each core computes logits[:, 4000k:4000k+4000].
  - Matmul streams in bf16; state kept in fp32 on-chip.

Gate packing: z tile is [128, 1024] per band m (partition = 32*m + b), free
col = gate*256 + 32*fc + r for unit u = 128*fc + 32*m + r, gates ordered
[g, i, f, o].  With this packing the DVE 32x32 block transpose of the h tile
directly yields h^T in natural u-major chunks (one copy per step).
Decoder state is scaled: H = 2*h, S = 2*c (folded into host-side weights).
Logits rows are ordered (core, t, local batch); the host unshards.
"""

import numpy as np

B, TS, TD = 32, 64, 63
VS, VT = 32000, 32000
E, U = 256, 1024
G4 = 4 * U
NB = 4            # batches per core
NC = 8            # cores
VSH = VT // NC    # vocab shard per core
RE = TS * NB      # encoder rows per core
RD = TD * NB      # decoder rows per core
RT = TD * B       # total decoder rows (all batches)

_GATE_PERM = [2, 0, 1, 3]  # new order [g, i, f, o] -> original gate index


def _reorder_cols(w):
    # natural col = gate_orig*1024 + u, u = 128*fc + 32*m + r
    w5 = w.reshape(w.shape[0], 4, 8, 4, 32)        # [in, g_orig, fc, m, r]
    w5 = w5[:, _GATE_PERM]                          # [in, g_new, fc, m, r]
    w5 = w5.transpose(0, 3, 1, 2, 4)                # [in, m, g_new, fc, r]
    return np.ascontiguousarray(w5.reshape(w.shape[0], G4))


def _reorder_bias(b):
    b5 = b.reshape(4, 8, 4, 32)[_GATE_PERM].transpose(2, 0, 1, 3)
    return np.ascontiguousarray(b5.reshape(1, G4))


def _prep_host(inputs):
    import ml_dtypes
    bf16 = ml_dtypes.bfloat16
    f32 = np.float32
    enc_in = np.asarray(inputs["encoder_input"])
    dec_in = np.asarray(inputs["decoder_input"])
    Wx_e = np.asarray(inputs["Wx_e"], f32)
    Wh_e = np.asarray(inputs["Wh_e"], f32)
    b_e = np.asarray(inputs["b_e"], f32)
    Wx_d = np.asarray(inputs["Wx_d"], f32)
    Wh_d = np.asarray(inputs["Wh_d"], f32)
    b_d = np.asarray(inputs["b_d"], f32)
    Wm = np.asarray(inputs["Wm"], f32)
    Wa = np.asarray(inputs["Wa"], f32)
    Wf = np.asarray(inputs["Wf"], f32)
    bfv = np.asarray(inputs["bf"], f32)

    Wxd_x = Wx_d[:E]
    Wxd_a = Wx_d[E:]
    Wa_h, Wa_c = Wa[:U], Wa[U:]

    shared = {
        "Wxe": _reorder_cols(Wx_e).astype(bf16),
        "Whe": _reorder_cols(Wh_e).astype(bf16),
        "Whcomb": _reorder_cols(0.5 * (Wh_d + Wa_h @ Wxd_a)).astype(bf16),
        "Wca": _reorder_cols(Wa_c @ Wxd_a).astype(bf16),
        "Whd0": _reorder_cols(0.5 * Wh_d).astype(bf16),
        "Wxdx": _reorder_cols(Wxd_x).astype(bf16),
        "Wm": (0.5 * Wm).astype(bf16),
        "WaH": (0.5 * Wa_h).astype(bf16),
        "WaC": np.ascontiguousarray(Wa_c.astype(bf16)),
        "be": _reorder_bias(b_e),
        "bd": _reorder_bias(b_d),
        "enc_emb": np.ascontiguousarray(np.asarray(inputs["enc_emb"], f32)),
        "dec_emb": np.ascontiguousarray(np.asarray(inputs["dec_emb"], f32)),
    }
    Wf_bf = Wf.astype(bf16)
    per_core = []
    for k in range(NC):
        eidx = enc_in[NB * k:NB * (k + 1)]
        didx = dec_in[NB * k:NB * (k + 1)]
        per_core.append({
            "enc_idx": np.ascontiguousarray(eidx.T.reshape(RE, 1).astype(np.int32)),
            "dec_idx": np.ascontiguousarray(didx.T.reshape(RD, 1).astype(np.int32)),
            "Wfs": np.ascontiguousarray(Wf_bf[:, VSH * k:VSH * (k + 1)]),
            "bfs": np.ascontiguousarray(bfv[VSH * k:VSH * (k + 1)].reshape(1, VSH)),
        })
    return shared, per_core


# ---------------------------------------------------------------------------

def _build_nc(stage="full", debug=False):
    import re as _re
    from contextlib import ExitStack
    import concourse.bass as bass
    import concourse.mybir as mybir
    import concourse.tile as tile
    from concourse import bacc
    from concourse.masks import make_identity

    dt = mybir.dt
    AF = mybir.ActivationFunctionType
    ALU = mybir.AluOpType
    AX = mybir.AxisListType
    f32, bf = dt.float32, dt.bfloat16

    nc = bacc.Bacc("TRN2", target_bir_lowering=False, debug=False, num_devices=NC)

    enc_idx = nc.dram_tensor("enc_idx", [RE, 1], dt.int32, kind="ExternalInput")
    dec_idx = nc.dram_tensor("dec_idx", [RD, 1], dt.int32, kind="ExternalInput")
    enc_emb = nc.dram_tensor("enc_emb", [VS, E], f32, kind="ExternalInput")
    dec_emb = nc.dram_tensor("dec_emb", [VT, E], f32, kind="ExternalInput")
    Wxe = nc.dram_tensor("Wxe", [E, G4], bf, kind="ExternalInput")
    Whe = nc.dram_tensor("Whe", [U, G4], bf, kind="ExternalInput")
    Whcomb = nc.dram_tensor("Whcomb", [U, G4], bf, kind="ExternalInput")
    Wca_t = nc.dram_tensor("Wca", [U, G4], bf, kind="ExternalInput")
    Whd0 = nc.dram_tensor("Whd0", [U, G4], bf, kind="ExternalInput")
    Wxdx = nc.dram_tensor("Wxdx", [E, G4], bf, kind="ExternalInput")
    Wm_t = nc.dram_tensor("Wm", [U, U], bf, kind="ExternalInput")
    WaH_t = nc.dram_tensor("WaH", [U, U], bf, kind="ExternalInput")
    WaC_t = nc.dram_tensor("WaC", [U, U], bf, kind="ExternalInput")
    Wfs = nc.dram_tensor("Wfs", [U, VSH], bf, kind="ExternalInput")
    bfs = nc.dram_tensor("bfs", [1, VSH], f32, kind="ExternalInput")
    be_t = nc.dram_tensor("be", [1, G4], f32, kind="ExternalInput")
    bd_t = nc.dram_tensor("bd", [1, G4], f32, kind="ExternalInput")

    logits = nc.dram_tensor("logits", [RT, VSH], f32, kind="ExternalOutput")

    dbg = {}
    if debug:
        dbg["memT"] = nc.dram_tensor("dbg_memT", [128, 8, TS, NB], bf, kind="ExternalOutput")
        dbg["c_enc"] = nc.dram_tensor("dbg_cenc", [128, 256], f32, kind="ExternalOutput")
        dbg["keysT"] = nc.dram_tensor("dbg_keysT", [128, 8, NB, TS], bf, kind="ExternalOutput")
        dbg["HallT"] = nc.dram_tensor("dbg_HallT", [128, 8, TD + 1, NB], bf, kind="ExternalOutput")
        dbg["alTall"] = nc.dram_tensor("dbg_alTall", [128, 2, TD, NB], bf, kind="ExternalOutput")
        dbg["MemWca"] = nc.dram_tensor("dbg_MemWca", [128, 2, G4], bf, kind="ExternalOutput")
        dbg["attnT"] = nc.dram_tensor("dbg_attnT", [128, 8, RD], bf, kind="ExternalOutput")

    with tile.TileContext(nc) as tc, ExitStack() as ctx:
        constp = ctx.enter_context(tc.tile_pool(name="const", bufs=1))
        ident = constp.tile([128, 128], bf)
        make_identity(nc, ident[:])

        dramp = ctx.enter_context(tc.tile_pool(name="dram", bufs=1, space="DRAM"))
        Xe_d = dramp.tile([RE, G4], bf, tag="Xe")
        Xd_d = dramp.tile([RD, G4], bf, tag="Xd")
        CHUNKS = [(0, 16), (16, 32), (32, 48), (48, TD)]
        aginC = [dramp.tile([8, 128, (c1 - c0) * NB], bf, tag=f"agin{j}",
                            name=f"aginC{j}")
                 for j, (c0, c1) in enumerate(CHUNKS)]
        agoutC = [dramp.tile([NC, 8, 128, (c1 - c0) * NB], bf, tag=f"agout{j}",
                             name=f"agoutC{j}", addr_space="Shared")
                  for j, (c0, c1) in enumerate(CHUNKS)]

        statep = ctx.enter_context(tc.tile_pool(name="state", bufs=1))
        memT = statep.tile([128, 8, TS, NB], bf)       # encoder h^T (true scale)
        c_sb = statep.tile([128, 256], f32)            # c (enc) / S=2c (dec)
        keysT = statep.tile([128, 8, NB, TS], bf)      # keys^T, batch-major
        HdecT = statep.tile([128, 8, TD + 1, NB], bf)  # slot t+1 = H_t = 2h_t
        alTall = statep.tile([128, 2, TD, NB], bf)     # block-diag align rows=(q,s), cols=b (other pair zero)
        MemWca = statep.tile([128, 2, G4], bf)         # (mem @ Wca), rows=(q,s)
        MemWaC = statep.tile([128, 2, U], bf)          # (mem @ Wa_c), rows=(q,s)
        attnT = statep.tile([128, 8, RD], bf)
        aT = statep.tile([128, 8, NC, TD, NB], bf)     # gathered activations

        gp = ctx.enter_context(tc.tile_pool(name="gates", bufs=1))
        xe_pp0 = gp.tile([128, 1024], bf, tag="xpp0")
        xe_pp1 = gp.tile([128, 1024], bf, tag="xpp1")
        xe_pp = [xe_pp0, xe_pp1]
        for i in range(2):
            nc.vector.memset(xe_pp[i][:], 0.0)
        z_sb = gp.tile([128, 1024], f32)
        t_g = gp.tile([128, 256], f32)
        s_i = gp.tile([128, 256], f32)
        s_f = gp.tile([128, 256], f32)
        s_o = gp.tile([128, 256], f32)
        tmp1 = gp.tile([128, 256], f32)
        tmp2 = gp.tile([128, 256], f32)
        tmp3 = gp.tile([128, 256], f32)
        tanh_c = gp.tile([128, 256], f32)
        h_bf = gp.tile([128, 256], bf)
        h_tr = gp.tile([128, 256], bf, tag="h_tr")

        # ------------- embedding gathers + X precomputes -------------
        # All gathers issue first (their HBM latency overlaps once), then the
        # PE transposes/matmuls and stores per 128-row tile.
        def x_precompute_all(jobs):
            with ExitStack() as c2:
                pp = c2.enter_context(tc.tile_pool(name="xpre", bufs=2))
                pp1 = c2.enter_context(tc.tile_pool(name="xpre1", bufs=1))
                psx = c2.enter_context(tc.tile_pool(name="xpre_ps", bufs=1, space="PSUM"))
                tiles = []
                for jj, (idx_t, emb_t, w_t, bias_t, rows, out_d) in enumerate(jobs):
                    nm = (rows + 127) // 128
                    for m in range(nm):
                        r0 = 128 * m
                        rr = min(128 * (m + 1), rows) - r0
                        idx_sb = pp1.tile([128, 1], dt.int32, name=f"idx{jj}_{m}")
                        nc.sync.dma_start(out=idx_sb[:rr, :], in_=idx_t[r0:r0 + rr, :])
                        gath = pp1.tile([128, E], f32, name=f"gath{jj}_{m}")
                        nc.gpsimd.indirect_dma_start(
                            out=gath[:rr, :], out_offset=None,
                            in_=emb_t[:],
                            in_offset=bass.IndirectOffsetOnAxis(ap=idx_sb[:rr, :1],
                                                                axis=0))
                        gbf = pp1.tile([128, E], bf, name=f"gbf{jj}_{m}")
                        nc.vector.tensor_copy(gbf[:rr, :], gath[:rr, :])
                        tiles.append((jj, r0, rr, gbf))
                # one shared weight/bias staging pair; jobs run sequentially
                w_sb = pp1.tile([128, 2, G4], bf, name="wx")
                bias_bc = pp1.tile([128, G4], f32, name="biasbc")
                cur = [None]

                def _stage_wb(jj):
                    w_t, bias_t = jobs[jj][2], jobs[jj][3]
                    for kk in range(2):
                        nc.scalar.dma_start(out=w_sb[:, kk, :],
                                            in_=w_t[128 * kk:128 * (kk + 1), :])
                    nc.scalar.dma_start(out=bias_bc[:],
                                        in_=bias_t[:].to_broadcast([128, G4]))
                    cur[0] = jj

                for jj, r0, rr, gbf in tiles:
                    if cur[0] != jj:
                        _stage_wb(jj)
                    out_d = jobs[jj][5]
                    xT = pp.tile([128, 2, 128], bf, tag="xT")
                    for kk in range(2):
                        pt = psx.tile([128, 128], bf, tag="ptr")
                        nc.tensor.transpose(pt[:, :rr], gbf[:rr, 128 * kk:128 * (kk + 1)],
                                            ident[:rr, :rr])
                        nc.vector.tensor_copy(xT[:, kk, :rr], pt[:, :rr])
                    for chv in range(8):
                        cs = 512 * chv
                        ps = psx.tile([128, 512], f32, tag="pmm")
                        for kk in range(2):
                            nc.tensor.matmul(ps[:rr, :], xT[:, kk, :rr],
                                             w_sb[:, kk, cs:cs + 512],
                                             start=(kk == 0), stop=(kk == 1))
                        st = pp.tile([128, 512], bf, tag="stage")
                        nc.vector.tensor_add(st[:rr, :], ps[:rr, :],
                                             bias_bc[:rr, cs:cs + 512])
                        nc.sync.dma_start(out=out_d[r0:r0 + rr, cs:cs + 512],
                                          in_=st[:rr, :])



        def load_x(dst, src_d, t):
            for m in range(4):
                nc.sync.dma_start(
                    out=dst[32 * m:32 * m + NB, :],
                    in_=src_d[NB * t:NB * (t + 1), 1024 * m:1024 * (m + 1)])

        def h_transpose(dst):
            # h_bf [128, 256] (row 32m+b, col 32fc+r; u=128fc+32m+r) -> dst [128, 8, NB]
            nc.vector.transpose(h_tr[:], h_bf[:])
            nc.vector.tensor_copy(
                dst, h_tr[:].rearrange("p (k c) -> p k c", k=8)[:, :, 0:NB])

        # ------------- scans (shared psum pool) -------------
        with ExitStack() as scn:
            psp = scn.enter_context(tc.tile_pool(name="scanps", bufs=1, space="PSUM"))
            psum_z0 = psp.tile([128, 1024], f32, tag="pz0")
            psum_z1 = psp.tile([128, 1024], f32, tag="pz1")
            psum_sc = psp.tile([128, 256], f32, tag="psc")
            psum_mw = psp.tile([128, 512], f32, tag="pmw")
            nc.vector.memset(psum_z0[:], 0.0)
            nc.vector.memset(psum_z1[:], 0.0)

            # ---------------- encoder ----------------
            with ExitStack() as c2:
                ep = c2.enter_context(tc.tile_pool(name="enc", bufs=1))
                whe_sb = ep.tile([128, 8, G4], bf)
                # weight loads ride the Scalar queue so they never block the
                # Sync queue's latency-critical x loads
                for kk in range(8):
                    nc.scalar.dma_start(out=whe_sb[:, kk, :],
                                        in_=Whe[128 * kk:128 * (kk + 1), :])

                x_precompute_all([
                    (enc_idx, enc_emb, Wxe, be_t, RE, Xe_d),
                    (dec_idx, dec_emb, Wxdx, bd_t, RD, Xd_d),
                ])

                load_x(xe_pp[0], Xe_d, 0)
                for t in range(TS):
                    xe_sb = xe_pp[t % 2]
                    if t + 1 < TS:
                        load_x(xe_pp[(t + 1) % 2], Xe_d, t + 1)
                    if t == 0:
                        zin = xe_sb
                        # gates chv0: g, i
                        nc.scalar.activation(t_g[:], zin[:, 0:256], AF.Tanh)
                        nc.scalar.activation(s_i[:], zin[:, 256:512], AF.Sigmoid)
                        nc.vector.tensor_mul(tmp2[:], s_i[:], t_g[:])
                        nc.scalar.activation(s_o[:], zin[:, 768:1024], AF.Sigmoid)
                        nc.vector.tensor_copy(c_sb[:], tmp2[:])
                    else:
                        zin = z_sb
                        for chv in range(2):
                            o0 = 512 * chv
                            for m in range(4):
                                co = 1024 * m + o0
                                for kk in range(8):
                                    nc.tensor.matmul(
                                        psum_z0[32 * m:32 * m + NB, o0:o0 + 512],
                                        memT[:, kk, t - 1, :],
                                        whe_sb[:, kk, co:co + 512],
                                        start=(kk == 0), stop=(kk == 7),
                                        tile_position=(0, 32 * m))
                            nc.vector.tensor_add(z_sb[:, o0:o0 + 512],
                                                 psum_z0[:, o0:o0 + 512],
                                                 xe_sb[:, o0:o0 + 512])
                            if chv == 0:
                                nc.scalar.activation(t_g[:], zin[:, 0:256], AF.Tanh)
                                nc.scalar.activation(s_i[:], zin[:, 256:512], AF.Sigmoid)
                                nc.vector.tensor_mul(tmp2[:], s_i[:], t_g[:])
                        nc.scalar.activation(s_f[:], zin[:, 512:768], AF.Sigmoid)
                        nc.scalar.activation(s_o[:], zin[:, 768:1024], AF.Sigmoid)
                        nc.vector.tensor_mul(tmp1[:], s_f[:], c_sb[:])
                        nc.vector.tensor_add(c_sb[:], tmp1[:], tmp2[:])
                    nc.scalar.activation(tanh_c[:], c_sb[:], AF.Tanh)
                    nc.vector.tensor_mul(h_bf[:], s_o[:], tanh_c[:])
                    h_transpose(memT[:, :, t, :])

                # keysT = (mem @ 0.5*Wm)^T, stored batch-major [p, kk, b, s]
                wm_sb = ep.tile([128, 8, U], bf)
                for kk in range(8):
                    nc.scalar.dma_start(out=wm_sb[:, kk, :],
                                        in_=Wm_t[128 * kk:128 * (kk + 1), :])
                for ko in range(8):
                    for kk in range(8):
                        nc.tensor.matmul(psum_mw[:, 0:256],
                                         wm_sb[:, kk, 128 * ko:128 * (ko + 1)],
                                         memT[:, kk, :, :],
                                         start=(kk == 0), stop=(kk == 7))
                    nc.vector.tensor_copy(
                        keysT[:, ko],
                        psum_mw[:, 0:256].rearrange("p (s b) -> p b s", b=NB))

                if debug:
                    nc.sync.dma_start(out=dbg["memT"][:], in_=memT[:])
                    nc.sync.dma_start(out=dbg["c_enc"][:], in_=c_sb[:])
                    nc.sync.dma_start(out=dbg["keysT"][:], in_=keysT[:])

            # ---------------- decoder precomputes ----------------
            m_dec = _re.match(r"dec(\d+)$", stage)
            TD_RUN = int(m_dec.group(1)) if m_dec else TD
            if stage != "enc":
                with ExitStack() as c2:
                    dp = c2.enter_context(tc.tile_pool(name="dec", bufs=1))
                    wah_sb = dp.tile([128, 8, U], bf)
                    for kk in range(8):
                        nc.scalar.dma_start(out=wah_sb[:, kk, :],
                                            in_=WaH_t[128 * kk:128 * (kk + 1), :])
                    # MemWca = mem @ Wca  (rows 64q+s for batch 2p+q)
                    with ExitStack() as c3:
                        wcap2 = c3.enter_context(tc.tile_pool(name="wca2", bufs=1))
                        # memQ[:, kk, p, 64q+s] = memT[:, kk, s, 2p+q]
                        memQ = wcap2.tile([128, 8, 2, 128], bf)
                        for kk in range(8):
                            for p in range(2):
                                nc.vector.tensor_copy(
                                    memQ[:, kk, p, :].rearrange("p (q s) -> p q s", q=2),
                                    memT[:, kk, :, 2 * p:2 * p + 2].rearrange(
                                        "p s q -> p q s"))
                        wca_sb = wcap2.tile([128, 8, G4], bf)
                        for kk in range(8):
                            nc.scalar.dma_start(out=wca_sb[:, kk, :],
                                                in_=Wca_t[128 * kk:128 * (kk + 1), :])
                        for p in range(2):
                            for c8 in range(8):
                                for kk in range(8):
                                    nc.tensor.matmul(
                                        psum_mw[:], memQ[:, kk, p, :],
                                        wca_sb[:, kk, 512 * c8:512 * (c8 + 1)],
                                        start=(kk == 0), stop=(kk == 7))
                                nc.vector.tensor_copy(
                                    MemWca[:, p, 512 * c8:512 * (c8 + 1)], psum_mw[:])
                        # MemWaC = mem @ Wa_c
                        wac_sb = wcap2.tile([128, 8, U], bf)
                        for kk in range(8):
                            nc.scalar.dma_start(out=wac_sb[:, kk, :],
                                                in_=WaC_t[128 * kk:128 * (kk + 1), :])
                        for p in range(2):
                            for c2_ in range(2):
                                for kk in range(8):
                                    nc.tensor.matmul(
                                        psum_mw[:], memQ[:, kk, p, :],
                                        wac_sb[:, kk, 512 * c2_:512 * (c2_ + 1)],
                                        start=(kk == 0), stop=(kk == 7))
                                nc.vector.tensor_copy(
                                    MemWaC[:, p, 512 * c2_:512 * (c2_ + 1)], psum_mw[:])

                    # ---------------- decoder scan ----------------
                    whcp = c2.enter_context(tc.tile_pool(name="whc", bufs=1))
                    whc_sb = whcp.tile([128, 8, G4], bf)
                    for kk in range(8):
                        nc.scalar.dma_start(out=whc_sb[:, kk, :],
                                            in_=Whcomb[128 * kk:128 * (kk + 1), :])
                    nc.vector.memset(alTall[:], 0.0)
                    nc.vector.tensor_scalar_mul(c_sb[:], c_sb[:], 2.0)
                    for kk in range(8):
                        nc.vector.tensor_scalar_mul(HdecT[:, kk, 0, :],
                                                    memT[:, kk, TS - 1, :], 2.0)

                    exp_sc = dp.tile([32, 256], f32)
                    rsums = dp.tile([32, NB], f32)
                    rmask = dp.tile([32, NB], f32)
                    rsD = dp.tile([32, 1], f32)
                    align_bf = dp.tile([32, 256], bf)
                    dve_t = dp.tile([32, 256], bf)
                    # rmask[p, b] = 1 iff p == b (diag selector)
                    nc.vector.tensor_copy(rmask[:], ident[0:32, 0:NB])

                    w0p = c2.enter_context(tc.tile_pool(name="w0", bufs=2))

                    # t=0 z-stream: H_enc @ Whd0 into psum_z0
                    load_x(xe_pp[0], Xd_d, 0)
                    for kk in range(8):
                        w0 = w0p.tile([128, G4], bf, tag="w0")
                        nc.scalar.dma_start(out=w0[:], in_=Whd0[128 * kk:128 * (kk + 1), :])
                        for chv in range(2):
                            o0 = 512 * chv
                            for m in range(4):
                                nc.tensor.matmul(
                                    psum_z0[32 * m:32 * m + NB, o0:o0 + 512],
                                    HdecT[:, kk, 0, :],
                                    w0[:, 1024 * m + o0:1024 * m + o0 + 512],
                                    start=(kk == 0), stop=(kk == 7),
                                    tile_position=(0, 32 * m))

                    psum_zp = [psum_z0, psum_z1]
                    for t in range(TD_RUN):
                        zp = psum_zp[t % 2]
                        zn = psum_zp[(t + 1) % 2]
                        xd_sb = xe_pp[t % 2]
                        if t + 1 < TD_RUN:
                            load_x(xe_pp[(t + 1) % 2], Xd_d, t + 1)
                        # gates (tanh identity; S=2c, H=2h), chv-split
                        nc.vector.tensor_add(z_sb[:, 0:512], zp[:, 0:512],
                                             xd_sb[:, 0:512])
                        nc.scalar.activation(t_g[:], z_sb[:, 0:256], AF.Tanh)
                        nc.scalar.activation(s_i[:], z_sb[:, 256:512], AF.Tanh, scale=0.5)
                        nc.vector.tensor_mul(tmp2[:], s_i[:], t_g[:])
                        nc.vector.tensor_add(tmp2[:], tmp2[:], t_g[:])
                        nc.vector.tensor_add(z_sb[:, 512:1024], zp[:, 512:1024],
                                             xd_sb[:, 512:1024])
                        nc.scalar.activation(s_f[:], z_sb[:, 512:768], AF.Tanh, scale=0.5)
                        nc.scalar.activation(s_o[:], z_sb[:, 768:1024], AF.Tanh, scale=0.5)
                        nc.vector.tensor_mul(tmp1[:], s_f[:], c_sb[:])
                        nc.vector.tensor_add(tmp1[:], tmp1[:], c_sb[:])
                        nc.vector.tensor_scalar_mul(tmp1[:], tmp1[:], 0.5)
                        nc.vector.tensor_add(c_sb[:], tmp1[:], tmp2[:])
                        nc.scalar.activation(tanh_c[:], c_sb[:], AF.Tanh, scale=0.5)
                        nc.vector.tensor_mul(tmp3[:], s_o[:], tanh_c[:])
                        nc.vector.tensor_add(h_bf[:], tmp3[:], tanh_c[:])
                        h_transpose(HdecT[:, :, t + 1, :])

                        # scores (PE): all batches at once, diagonal blocks used
                        for kk in range(8):
                            nc.tensor.matmul(
                                psum_sc[0:NB, :],
                                HdecT[:, kk, t + 1, :],
                                keysT[:, kk].rearrange("p b s -> p (b s)"),
                                start=(kk == 0), stop=(kk == 7))

                        # z_{t+1} Whcomb stream (PE), needs H_t only
                        if t + 1 < TD_RUN:
                            for chv in range(2):
                                o0 = 512 * chv
                                for m in range(4):
                                    co = 1024 * m + o0
                                    for kk in range(8):
                                        nc.tensor.matmul(
                                            zn[32 * m:32 * m + NB, o0:o0 + 512],
                                            HdecT[:, kk, t + 1, :],
                                            whc_sb[:, kk, co:co + 512],
                                            start=(kk == 0), stop=False,
                                            tile_position=(0, 32 * m))

                        # softmax + align transpose (vector/scalar).
                        # psum_sc rows 0..3 hold cross-batch scores [b, (b', s)];
                        # only the diagonal blocks b'==b are used.
                        nc.scalar.activation(exp_sc[:], psum_sc[0:32, :], AF.Exp)
                        for b in range(NB):
                            nc.vector.reduce_sum(rsums[:, b:b + 1],
                                                 exp_sc[:, 64 * b:64 * (b + 1)],
                                                 axis=AX.X)
                        # rsD[p] = rsums[p, p] via identity-mask multiply + reduce
                        nc.vector.tensor_mul(rsums[:], rsums[:], rmask[:])
                        nc.vector.reduce_sum(rsD[:], rsums[:], axis=AX.X)
                        nc.vector.reciprocal(rsD[:], rsD[:])
                        nc.vector.tensor_scalar(align_bf[:], exp_sc[:],
                                                rsD[:, 0:1], None, op0=ALU.mult)
                        nc.vector.transpose(dve_t[:], align_bf[:])
                        # diag value align_b[32h+r] sits at dve_t[r, 32*(2b+h)+b]
                        for b in range(NB):
                            p, q = b // 2, b % 2
                            for hh in range(2):
                                cc = 32 * (2 * b + hh) + b
                                nc.vector.tensor_copy(
                                    alTall[64 * q + 32 * hh:64 * q + 32 * hh + 32,
                                           p, t, b:b + 1],
                                    dve_t[0:32, cc:cc + 1])

                        # align part of z_{t+1} (PE; emitted after the alTall
                        # writes so the dependency points the right way)
                        if t + 1 < TD_RUN:
                            for chv in range(2):
                                o0 = 512 * chv
                                for m in range(4):
                                    co = 1024 * m + o0
                                    for p in range(2):
                                        nc.tensor.matmul(
                                            zn[32 * m:32 * m + NB, o0:o0 + 512],
                                            alTall[:, p, t, :],
                                            MemWca[:, p, co:co + 512],
                                            start=False, stop=(p == 1),
                                            tile_position=(0, 32 * m))

                        # chunked attention output + AllGather, overlapped with
                        # the remaining decoder steps
                        if stage == "full" and (t + 1) in [c1 for _, c1 in CHUNKS]:
                            j = [c1 for _, c1 in CHUNKS].index(t + 1)
                            c0, c1 = CHUNKS[j]
                            cw = (c1 - c0) * NB
                            for ko in range(8):
                                pa = psum_mw[:, 0:cw]
                                for kk in range(8):
                                    nc.tensor.matmul(
                                        pa, wah_sb[:, kk, 128 * ko:128 * (ko + 1)],
                                        HdecT[:, kk, 1 + c0:1 + c1, :],
                                        start=(kk == 0), stop=False)
                                for p in range(2):
                                    nc.tensor.matmul(
                                        pa,
                                        MemWaC[:, p, 128 * ko:128 * (ko + 1)],
                                        alTall[:, p, c0:c1, :].rearrange(
                                            "p t b -> p (t b)"),
                                        start=False, stop=(p == 1))
                                nc.vector.tensor_copy(
                                    attnT[:, ko, NB * c0:NB * c1], pa)
                            nc.gpsimd.dma_start(
                                out=aginC[j][:].rearrange("k p c -> p k c"),
                                in_=attnT[:, :, NB * c0:NB * c1])
                            nc.gpsimd.collective_compute(
                                "AllGather", ALU.bypass,
                                ins=[aginC[j][:]], outs=[agoutC[j][:]],
                                replica_groups=[list(range(NC))])
                            for kk in range(8):
                                for r in range(NC):
                                    nc.gpsimd.dma_start(out=aT[:, kk, r, c0:c1, :],
                                                        in_=agoutC[j][r, kk])

                    if debug:
                        nc.sync.dma_start(out=dbg["HallT"][:], in_=HdecT[:])
                        nc.sync.dma_start(out=dbg["alTall"][:], in_=alTall[:])
                        nc.sync.dma_start(out=dbg["MemWca"][:], in_=MemWca[:])

        # ------- projection (aT filled by the chunked AllGather above) -------
        if stage == "full":
            with ExitStack() as c2:
                pp = c2.enter_context(tc.tile_pool(name="proj", bufs=1))
                ppd = c2.enter_context(tc.tile_pool(name="projd", bufs=3))
                ps4 = c2.enter_context(tc.tile_pool(name="projps", bufs=8, space="PSUM"))
                if debug:
                    nc.sync.dma_start(out=dbg["attnT"][:], in_=attnT[:])
                aTf = aT[:].rearrange("p k r t b -> p k (r t b)")
                nmt = (RT + 127) // 128
                NCH = VSH // 500
                wfp = c2.enter_context(tc.tile_pool(name="wfc", bufs=2))
                for sc in range(NCH):
                    wf_c = wfp.tile([128, 8, 500], bf, tag="wfc")
                    for kk in range(8):
                        nc.scalar.dma_start(
                            out=wf_c[:, kk, :],
                            in_=Wfs[128 * kk:128 * (kk + 1), 500 * sc:500 * (sc + 1)])
                    bfc = wfp.tile([128, 500], f32, tag="bfc")
                    nc.scalar.dma_start(
                        out=bfc[:],
                        in_=bfs[:, 500 * sc:500 * (sc + 1)].to_broadcast([128, 500]))
                    for m in range(nmt):
                        r0 = 128 * m
                        rr = min(128 * (m + 1), RT) - r0
                        pj = ps4.tile([128, 500], f32, tag="pj")
                        for kk in range(8):
                            nc.tensor.matmul(pj[:rr, :], aTf[:, kk, r0:r0 + rr],
                                             wf_c[:, kk, :],
                                             start=(kk == 0), stop=(kk == 7))
                        st = ppd.tile([128, 500], f32, tag="st")
                        nc.vector.tensor_add(st[:rr, :], pj[:rr, :], bfc[:rr, :])
                        nc.sync.dma_start(out=logits[r0:r0 + rr, 500 * sc:500 * (sc + 1)],
                                          in_=st[:rr, :])
        else:
            # partial-stage dummy output so the NEFF has its ExternalOutput written
            st0 = gp.tile([1, 4], f32, tag="dummy")
            nc.vector.tensor_copy(st0[:], z_sb[0:1, 0:4])
            nc.sync.dma_start(out=logits[0:1, 0:4], in_=st0[:])

    nc.finalize()
    return nc, dbg


_CACHE = {}


def _get_nc(stage="full", debug=False):
    key = (stage, debug)
    if key not in _CACHE:
        _CACHE[key] = _build_nc(stage, debug)
    return _CACHE[key]


def run_cores(inputs, stage="full", debug=False, trace=False):
    from concourse.bass_utils import run_bass_kernel_spmd
    shared, per_core = _prep_host(inputs)
    nc, dbg = _get_nc(stage, debug)
    in_maps = []
    for k in range(NC):
        m = dict(shared)
        m.update(per_core[k])
        in_maps.append(m)
    return run_bass_kernel_spmd(nc, in_maps, core_ids=list(range(NC)), trace=trace)


def unshard(outs):
    full = np.concatenate(outs, axis=1)                     # [2016, 32000]
    # rows ordered (r, t, b_local); batch b = 4*r + b_local
    full = full.reshape(NC, TD, NB, VT).transpose(0, 2, 1, 3).reshape(B, TD, VT)
    return np.ascontiguousarray(full.astype(np.float32))


def kernel(**inputs):
    res = run_cores(inputs, stage="full")
    outs = [np.asarray(r["logits"]) for r in res.results]   # [2016, 4000] each
    return unshard(outs)



# revision 17
# speedup vs baseline: 1.0534x; 1.0534x over previous
"""Trainium2 Bass kernel for nn_DmTranslateTrain (seq2seq translate train step).

Strategy (8 NeuronCores, SPMD):
  - Data-parallel over batch: core k owns batches [4k, 4k+4). Each core runs the
    full encoder LSTM scan + decoder (LSTM + Luong attention) for its 4 batches.
  - Output projection is tensor-parallel over the vocabulary: chunked AllGather
    of attention activations overlapped with the decoder, then each core
    computes logits[:, 4000k:4000k+4000].

Scan-step design (the hot loop):
  - The x-projection (emb @ Wx + b, precomputed in DRAM) is folded into the PE
    accumulation with a tiny K=4 identity matmul, so the gate nonlinearities
    read PSUM directly (no vector adds on the critical path).
  - All four gates use plain tanh: sigma(x) = (1+tanh(x/2))/2, with the 0.5
    pre-scale for gates i/f/o folded into the weight columns host-side.  One
    fused tanh per 512-col gate pair (2 ACTs per step), one activation table.
  - State kept scaled: C2 = 2c, H = 2h.  Updates via scalar_tensor_tensor:
      IG2 = (ti+1)*tu; FC2 = (tf+1)*C2; C2' = 0.5*FC2 + IG2;
      tc = tanh(0.5*C2'); H = (to+1)*tc.
  - z matmuls emitted kk-outer / m-inner so the 4 PE column-groups
    (tile_position=(0,32m)) stream concurrently; decoder scores run in column
    group q96 on separate PSUM partitions.

Gate packing: z tile is [128, 1024] per band m (partition = 32*m + b), free
col = gate*256 + 32*fc + r for unit u = 128*fc + 32*m + r, gates ordered
[u, i, f, o] (u = candidate).  The DVE 32x32 block transpose of the H tile
directly yields H^T in natural u-major chunks (one copy per step).
Logits rows are ordered (core, t, local batch); the host unshards.
"""

import numpy as np

B, TS, TD = 32, 64, 63
VS, VT = 32000, 32000
E, U = 256, 1024
G4 = 4 * U
NB = 4            # batches per core
NC = 8            # cores
VSH = VT // NC    # vocab shard per core
RE = TS * NB      # encoder rows per core
RD = TD * NB      # decoder rows per core
RT = TD * B       # total decoder rows (all batches)

_GATE_PERM = [2, 0, 1, 3]  # new order [u, i, f, o] -> original gate index
CHUNKS = [(0, 16), (16, 32), (32, 48), (48, 60), (60, TD)]


def _reorder_cols(w):
    # natural col = gate_orig*1024 + u, u = 128*fc + 32*m + r
    w5 = w.reshape(w.shape[0], 4, 8, 4, 32)        # [in, g_orig, fc, m, r]
    w5 = w5[:, _GATE_PERM]                          # [in, g_new, fc, m, r]
    w5 = w5.transpose(0, 3, 1, 2, 4)                # [in, m, g_new, fc, r]
    return np.ascontiguousarray(w5.reshape(w.shape[0], G4))


def _reorder_bias(b):
    b5 = b.reshape(4, 8, 4, 32)[_GATE_PERM].transpose(2, 0, 1, 3)
    return np.ascontiguousarray(b5.reshape(1, G4))


def _prep_host(inputs):
    import ml_dtypes
    bf16 = ml_dtypes.bfloat16
    f32 = np.float32
    enc_in = np.asarray(inputs["encoder_input"])
    dec_in = np.asarray(inputs["decoder_input"])
    Wx_e = np.asarray(inputs["Wx_e"], f32)
    Wh_e = np.asarray(inputs["Wh_e"], f32)
    b_e = np.asarray(inputs["b_e"], f32)
    Wx_d = np.asarray(inputs["Wx_d"], f32)
    Wh_d = np.asarray(inputs["Wh_d"], f32)
    b_d = np.asarray(inputs["b_d"], f32)
    Wm = np.asarray(inputs["Wm"], f32)
    Wa = np.asarray(inputs["Wa"], f32)
    Wf = np.asarray(inputs["Wf"], f32)
    bfv = np.asarray(inputs["bf"], f32)

    Wxd_x = Wx_d[:E]
    Wxd_a = Wx_d[E:]
    Wa_h, Wa_c = Wa[:U], Wa[U:]

    # per-gate column scale on the NATURAL layout (i, f, g, o): tanh trick
    # needs 0.5*z for i/f/o; the candidate gate g keeps full scale.
    cs = np.concatenate([np.full(U, 0.5, f32), np.full(U, 0.5, f32),
                         np.ones(U, f32), np.full(U, 0.5, f32)])

    shared = {
        "Wxe": _reorder_cols(Wx_e * cs).astype(bf16),
        "Whe": _reorder_cols(0.5 * Wh_e * cs).astype(bf16),
        "Whcomb": _reorder_cols(0.5 * (Wh_d + Wa_h @ Wxd_a) * cs).astype(bf16),
        "Wca": _reorder_cols(0.5 * (Wa_c @ Wxd_a) * cs).astype(bf16),
        "Whd0": _reorder_cols(0.5 * Wh_d * cs).astype(bf16),
        "Wxdx": _reorder_cols(Wxd_x * cs).astype(bf16),
        "Wm": (0.25 * Wm).astype(bf16),
        "WaH": (0.5 * Wa_h).astype(bf16),
        "WaC": np.ascontiguousarray((0.5 * Wa_c).astype(bf16)),
        "be": _reorder_bias(b_e * cs),
        "bd": _reorder_bias(b_d * cs),
        "enc_emb": np.ascontiguousarray(np.asarray(inputs["enc_emb"], f32)),
        "dec_emb": np.ascontiguousarray(np.asarray(inputs["dec_emb"], f32)),
    }
    Wf_bf = Wf.astype(bf16)
    per_core = []
    for k in range(NC):
        eidx = enc_in[NB * k:NB * (k + 1)]
        didx = dec_in[NB * k:NB * (k + 1)]
        per_core.append({
            "enc_idx": np.ascontiguousarray(eidx.T.reshape(RE, 1).astype(np.int32)),
            "dec_idx": np.ascontiguousarray(didx.T.reshape(RD, 1).astype(np.int32)),
            "Wfs": np.ascontiguousarray(Wf_bf[:, VSH * k:VSH * (k + 1)]),
            "bfs": np.ascontiguousarray(bfv[VSH * k:VSH * (k + 1)].reshape(1, VSH)),
        })
    return shared, per_core


# ---------------------------------------------------------------------------

def _build_nc(stage="full", debug=False):
    import re as _re
    from contextlib import ExitStack
    import concourse.bass as bass
    import concourse.mybir as mybir
    import concourse.tile as tile
    from concourse import bacc
    from concourse.masks import make_identity

    dt = mybir.dt
    AF = mybir.ActivationFunctionType
    ALU = mybir.AluOpType
    AX = mybir.AxisListType
    f32, bf = dt.float32, dt.bfloat16

    nc = bacc.Bacc("TRN2", target_bir_lowering=False, debug=False, num_devices=NC)

    enc_idx = nc.dram_tensor("enc_idx", [RE, 1], dt.int32, kind="ExternalInput")
    dec_idx = nc.dram_tensor("dec_idx", [RD, 1], dt.int32, kind="ExternalInput")
    enc_emb = nc.dram_tensor("enc_emb", [VS, E], f32, kind="ExternalInput")
    dec_emb = nc.dram_tensor("dec_emb", [VT, E], f32, kind="ExternalInput")
    Wxe = nc.dram_tensor("Wxe", [E, G4], bf, kind="ExternalInput")
    Whe = nc.dram_tensor("Whe", [U, G4], bf, kind="ExternalInput")
    Whcomb = nc.dram_tensor("Whcomb", [U, G4], bf, kind="ExternalInput")
    Wca_t = nc.dram_tensor("Wca", [U, G4], bf, kind="ExternalInput")
    Whd0 = nc.dram_tensor("Whd0", [U, G4], bf, kind="ExternalInput")
    Wxdx = nc.dram_tensor("Wxdx", [E, G4], bf, kind="ExternalInput")
    Wm_t = nc.dram_tensor("Wm", [U, U], bf, kind="ExternalInput")
    WaH_t = nc.dram_tensor("WaH", [U, U], bf, kind="ExternalInput")
    WaC_t = nc.dram_tensor("WaC", [U, U], bf, kind="ExternalInput")
    Wfs = nc.dram_tensor("Wfs", [U, VSH], bf, kind="ExternalInput")
    bfs = nc.dram_tensor("bfs", [1, VSH], f32, kind="ExternalInput")
    be_t = nc.dram_tensor("be", [1, G4], f32, kind="ExternalInput")
    bd_t = nc.dram_tensor("bd", [1, G4], f32, kind="ExternalInput")

    logits = nc.dram_tensor("logits", [RT, VSH], f32, kind="ExternalOutput")

    dbg = {}
    if debug:
        dbg["memT"] = nc.dram_tensor("dbg_memT", [128, 8, TS, NB], bf, kind="ExternalOutput")
        dbg["c_enc"] = nc.dram_tensor("dbg_cenc", [128, 256], f32, kind="ExternalOutput")
        dbg["keysT"] = nc.dram_tensor("dbg_keysT", [128, 8, NB, TS], bf, kind="ExternalOutput")
        dbg["HallT"] = nc.dram_tensor("dbg_HallT", [128, 8, TD + 1, NB], bf, kind="ExternalOutput")
        dbg["alTall"] = nc.dram_tensor("dbg_alTall", [128, 2, TD, NB], bf, kind="ExternalOutput")
        dbg["MemWca"] = nc.dram_tensor("dbg_MemWca", [128, 2, G4], bf, kind="ExternalOutput")

    with tile.TileContext(nc) as tc, ExitStack() as ctx:
        constp = ctx.enter_context(tc.tile_pool(name="const", bufs=1))
        ident = constp.tile([128, 128], bf)
        make_identity(nc, ident[:])

        dramp = ctx.enter_context(tc.tile_pool(name="dram", bufs=1, space="DRAM"))
        Xe_d = dramp.tile([RE, G4], bf, tag="Xe")
        Xd_d = dramp.tile([RD, G4], bf, tag="Xd")
        aginC = [dramp.tile([8, 128, (c1 - c0) * NB], bf, tag=f"agin{j}",
                            name=f"aginC{j}")
                 for j, (c0, c1) in enumerate(CHUNKS)]
        agoutC = [dramp.tile([NC, 8, 128, (c1 - c0) * NB], bf, tag=f"agout{j}",
                             name=f"agoutC{j}", addr_space="Shared")
                  for j, (c0, c1) in enumerate(CHUNKS)]

        statep = ctx.enter_context(tc.tile_pool(name="state", bufs=1))
        memT = statep.tile([128, 8, TS, NB], bf)       # encoder H^T (= 2h)
        C2 = statep.tile([128, 256], f32)              # 2c (enc then dec)
        keysT = statep.tile([128, 8, NB, TS], bf)      # keys^T, batch-major
        HdecT = statep.tile([128, 8, TD + 1, NB], bf)  # slot t+1 = H_t = 2h_t
        alTall = statep.tile([128, 2, TD, NB], bf)     # block-diag align rows=(q,s), cols=b
        MemWca = statep.tile([128, 2, G4], bf)         # (memT @ Wca'), rows=(q,s)
        MemWaC = statep.tile([128, 2, U], bf)          # (memT @ WaC'), rows=(q,s)

        gp = ctx.enter_context(tc.tile_pool(name="gates", bufs=1))
        xe_pp = [gp.tile([NB, G4], bf, name=f"xe{i}") for i in range(2)]
        tga = gp.tile([128, 512], f32)   # tanh(z_u), tanh(z_i/2)
        tfo = gp.tile([128, 512], f32)   # tanh(z_f/2), tanh(z_o/2)
        IG2 = gp.tile([128, 256], f32)
        FC2 = gp.tile([128, 256], f32)
        tc_t = gp.tile([128, 256], f32)
        Hbf = gp.tile([128, 256], bf)
        h_tr = gp.tile([128, 256], bf, tag="h_tr")

        # ------------- embedding gathers + X precomputes -------------
        def x_precompute_all(jobs):
            with ExitStack() as c2:
                pp = c2.enter_context(tc.tile_pool(name="xpre", bufs=2))
                pp1 = c2.enter_context(tc.tile_pool(name="xpre1", bufs=1))
                psx = c2.enter_context(tc.tile_pool(name="xpre_ps", bufs=1, space="PSUM"))
                tiles = []
                for jj, (idx_t, emb_t, w_t, bias_t, rows, out_d) in enumerate(jobs):
                    nm = (rows + 127) // 128
                    for m in range(nm):
                        r0 = 128 * m
                        rr = min(128 * (m + 1), rows) - r0
                        idx_sb = pp1.tile([128, 1], dt.int32, name=f"idx{jj}_{m}")
                        nc.sync.dma_start(out=idx_sb[:rr, :], in_=idx_t[r0:r0 + rr, :])
                        gath = pp1.tile([128, E], f32, name=f"gath{jj}_{m}")
                        nc.gpsimd.indirect_dma_start(
                            out=gath[:rr, :], out_offset=None,
                            in_=emb_t[:],
                            in_offset=bass.IndirectOffsetOnAxis(ap=idx_sb[:rr, :1],
                                                                axis=0))
                        gbf = pp1.tile([128, E], bf, name=f"gbf{jj}_{m}")
                        nc.vector.tensor_copy(gbf[:rr, :], gath[:rr, :])
                        tiles.append((jj, r0, rr, gbf))
                w_sb = pp1.tile([128, 2, G4], bf, name="wx")
                bias_bc = pp1.tile([128, G4], f32, name="biasbc")
                cur = [None]

                def _stage_wb(jj):
                    w_t, bias_t = jobs[jj][2], jobs[jj][3]
                    for kk in range(2):
                        nc.scalar.dma_start(out=w_sb[:, kk, :],
                                            in_=w_t[128 * kk:128 * (kk + 1), :])
                    nc.scalar.dma_start(out=bias_bc[:],
                                        in_=bias_t[:].to_broadcast([128, G4]))
                    cur[0] = jj

                for jj, r0, rr, gbf in tiles:
                    if cur[0] != jj:
                        _stage_wb(jj)
                    out_d = jobs[jj][5]
                    xT = pp.tile([128, 2, 128], bf, tag="xT")
                    for kk in range(2):
                        pt = psx.tile([128, 128], bf, tag="ptr")
                        nc.tensor.transpose(pt[:, :rr], gbf[:rr, 128 * kk:128 * (kk + 1)],
                                            ident[:rr, :rr])
                        nc.vector.tensor_copy(xT[:, kk, :rr], pt[:, :rr])
                    for chv in range(8):
                        cs0 = 512 * chv
                        ps = psx.tile([128, 512], f32, tag="pmm")
                        for kk in range(2):
                            nc.tensor.matmul(ps[:rr, :], xT[:, kk, :rr],
                                             w_sb[:, kk, cs0:cs0 + 512],
                                             start=(kk == 0), stop=(kk == 1))
                        st = pp.tile([128, 512], bf, tag="stage")
                        nc.vector.tensor_add(st[:rr, :], ps[:rr, :],
                                             bias_bc[:rr, cs0:cs0 + 512])
                        nc.sync.dma_start(out=out_d[r0:r0 + rr, cs0:cs0 + 512],
                                          in_=st[:rr, :])

        def h_transpose(dst):
            # Hbf [128, 256] (row 32m+b, col 32fc+r; u=128fc+32m+r) -> dst [128, 8, NB]
            nc.vector.transpose(h_tr[:], Hbf[:])
            nc.vector.tensor_copy(
                dst, h_tr[:].rearrange("p (k c) -> p k c", k=8)[:, :, 0:NB])

        def gate_tail(ps, dst):
            # z in psum ps [128, 1024]; writes H^T into dst [128, 8, NB],
            # updates C2 in place.
            nc.scalar.activation(tga[:], ps[:, 0:512], AF.Tanh)
            nc.scalar.activation(tfo[:], ps[:, 512:1024], AF.Tanh)
            nc.vector.scalar_tensor_tensor(IG2[:], tga[:, 256:512], 1.0,
                                           tga[:, 0:256], op0=ALU.add, op1=ALU.mult)
            nc.vector.scalar_tensor_tensor(FC2[:], tfo[:, 0:256], 1.0,
                                           C2[:], op0=ALU.add, op1=ALU.mult)
            nc.vector.scalar_tensor_tensor(C2[:], FC2[:], 0.5,
                                           IG2[:], op0=ALU.mult, op1=ALU.add)
            nc.scalar.activation(tc_t[:], C2[:], AF.Tanh, scale=0.5)
            nc.vector.scalar_tensor_tensor(Hbf[:], tfo[:, 256:512], 1.0,
                                           tc_t[:], op0=ALU.add, op1=ALU.mult)
            h_transpose(dst)

        # ------------- scans (shared psum pool) -------------
        with ExitStack() as scn:
            psp = scn.enter_context(tc.tile_pool(name="scanps", bufs=1, space="PSUM"))
            psum_z0 = psp.tile([128, 1024], f32, tag="pz0")
            psum_z1 = psp.tile([128, 1024], f32, tag="pz1")
            psum_zp = [psum_z0, psum_z1]
            psum_sc = psp.tile([128, 256], f32, tag="psc")
            psum_mw = psp.tile([128, 512], f32, tag="pmw")

            def emit_z_stream(ps, xe, lhsT_of_kk, w_sb_of_kk, first, with_align,
                              al_t=None, kk_outer=False):
                # identity matmuls fold the x projection into psum
                for m in range(4):
                    for chv in range(2):
                        co = 1024 * m + 512 * chv
                        nc.tensor.matmul(
                            ps[32 * m:32 * m + NB, 512 * chv:512 * chv + 512],
                            ident[0:NB, 0:NB], xe[0:NB, co:co + 512],
                            start=True, stop=first,
                            tile_position=(0, 32 * m))
                if first:
                    return
                order = ([(chv, kk) for kk in range(8) for chv in range(2)]
                         if kk_outer else
                         [(chv, kk) for chv in range(2) for kk in range(8)])
                for chv, kk in order:
                    lh = lhsT_of_kk(kk)
                    for m in range(4):
                        co = 1024 * m + 512 * chv
                        nc.tensor.matmul(
                            ps[32 * m:32 * m + NB, 512 * chv:512 * chv + 512],
                            lh, w_sb_of_kk(kk)[:, co:co + 512],
                            start=False,
                            stop=(kk == 7 and not with_align),
                            tile_position=(0, 32 * m))
                if with_align:
                    for chv in range(2):
                        for m in range(4):
                            co = 1024 * m + 512 * chv
                            for p in range(2):
                                nc.tensor.matmul(
                                    ps[32 * m:32 * m + NB, 512 * chv:512 * chv + 512],
                                    alTall[:, p, al_t, :],
                                    MemWca[:, p, co:co + 512],
                                    start=False, stop=(p == 1),
                                    tile_position=(0, 32 * m))

            # x precompute first: its staging pools need the space the big
            # weight pools occupy later.
            x_precompute_all([
                (enc_idx, enc_emb, Wxe, be_t, RE, Xe_d),
                (dec_idx, dec_emb, Wxdx, bd_t, RD, Xd_d),
            ])

            # Whcomb is prefetched during the encoder (gpsimd queue is idle);
            # its pool lives for the whole scan scope.
            whcp = scn.enter_context(tc.tile_pool(name="whc", bufs=1))
            whc_sb = whcp.tile([128, 8, G4], bf)

            # ---------------- encoder ----------------
            with ExitStack() as ec:
                encp = ec.enter_context(tc.tile_pool(name="enc", bufs=1))
                whe_sb = encp.tile([128, 8, G4], bf)
                for kk in range(8):
                    nc.scalar.dma_start(out=whe_sb[:, kk, :],
                                        in_=Whe[128 * kk:128 * (kk + 1), :])

                nc.vector.memset(C2[:], 0.0)

                nc.sync.dma_start(out=xe_pp[0][:], in_=Xe_d[0:NB, :])
                for t in range(TS):
                    xe = xe_pp[t % 2]
                    ps = psum_zp[t % 2]
                    if t + 1 < TS:
                        nc.sync.dma_start(out=xe_pp[(t + 1) % 2][:],
                                          in_=Xe_d[NB * (t + 1):NB * (t + 2), :])
                    emit_z_stream(ps, xe,
                                  (lambda kk, _t=t: memT[:, kk, _t - 1, :]),
                                  (lambda kk: whe_sb[:, kk, :]),
                                  first=(t == 0), with_align=False)
                    gate_tail(ps, memT[:, :, t, :])
                    # prefetch Whcomb on the idle gpsimd queue
                    if stage != "enc" and t == 40:
                        for kk in range(8):
                            nc.gpsimd.dma_start(
                                out=whc_sb[:, kk, :],
                                in_=Whcomb[128 * kk:128 * (kk + 1), :])

                if debug:
                    nc.sync.dma_start(out=dbg["memT"][:], in_=memT[:])
                    nc.sync.dma_start(out=dbg["c_enc"][:], in_=C2[:])

            # ---------------- transition: keys, MemWca, MemWaC ----------------
            m_dec = _re.match(r"dec(\d+)$", stage)
            TD_RUN = int(m_dec.group(1)) if m_dec else TD
            if stage != "enc":
                decp = scn.enter_context(tc.tile_pool(name="dec", bufs=1))

                memQ = decp.tile([128, 8, 2, 128], bf)

                with ExitStack() as c3:
                    wmp = c3.enter_context(tc.tile_pool(name="wmp", bufs=1))
                    wm_sb = wmp.tile([128, 8, U], bf)
                    for kk in range(8):
                        nc.gpsimd.dma_start(out=wm_sb[:, kk, :],
                                            in_=Wm_t[128 * kk:128 * (kk + 1), :])
                    # keysT = (memT @ Wm')^T, stored batch-major [p, kk, b, s]
                    for ko in range(8):
                        for kk in range(8):
                            nc.tensor.matmul(psum_mw[:, 0:256],
                                             wm_sb[:, kk, 128 * ko:128 * (ko + 1)],
                                             memT[:, kk, :, :],
                                             start=(kk == 0), stop=(kk == 7))
                        nc.vector.tensor_copy(
                            keysT[:, ko],
                            psum_mw[:, 0:256].rearrange("p (s b) -> p b s", b=NB))

                    # memQ[:, kk, p, 64q+s] = memT[:, kk, s, 2p+q]
                    for kk in range(8):
                        for p in range(2):
                            nc.vector.tensor_copy(
                                memQ[:, kk, p, :].rearrange("p (q s) -> p q s", q=2),
                                memT[:, kk, :, 2 * p:2 * p + 2].rearrange(
                                    "p s q -> p q s"))

                with ExitStack() as c3b:
                    wcap2 = c3b.enter_context(tc.tile_pool(name="wca2", bufs=1))
                    wca_sb = wcap2.tile([128, 8, G4], bf)
                    for kk in range(8):
                        nc.gpsimd.dma_start(out=wca_sb[:, kk, :],
                                            in_=Wca_t[128 * kk:128 * (kk + 1), :])
                    for p in range(2):
                        for c8 in range(8):
                            for kk in range(8):
                                nc.tensor.matmul(
                                    psum_mw[:], memQ[:, kk, p, :],
                                    wca_sb[:, kk, 512 * c8:512 * (c8 + 1)],
                                    start=(kk == 0), stop=(kk == 7))
                            nc.vector.tensor_copy(
                                MemWca[:, p, 512 * c8:512 * (c8 + 1)], psum_mw[:])

                with ExitStack() as c3c:
                    wacp = c3c.enter_context(tc.tile_pool(name="wacp", bufs=1))
                    wac_sb = wacp.tile([128, 8, U], bf)
                    for kk in range(8):
                        nc.gpsimd.dma_start(out=wac_sb[:, kk, :],
                                            in_=WaC_t[128 * kk:128 * (kk + 1), :])
                    for p in range(2):
                        for c2_ in range(2):
                            for kk in range(8):
                                nc.tensor.matmul(
                                    psum_mw[:], memQ[:, kk, p, :],
                                    wac_sb[:, kk, 512 * c2_:512 * (c2_ + 1)],
                                    start=(kk == 0), stop=(kk == 7))
                            nc.vector.tensor_copy(
                                MemWaC[:, p, 512 * c2_:512 * (c2_ + 1)], psum_mw[:])

                if debug:
                    nc.sync.dma_start(out=dbg["keysT"][:], in_=keysT[:])
                    nc.sync.dma_start(out=dbg["MemWca"][:], in_=MemWca[:])

                # ---------------- decoder scan ----------------
                nc.vector.memset(alTall[:], 0.0)

                rsums = decp.tile([128, NB], f32)
                rmask = decp.tile([128, NB], f32)
                rsD = decp.tile([128, 1], f32)
                # rmask[96+p, b] = 1 iff p == b (diag selector)
                nc.vector.tensor_copy(rmask[96:128, :], ident[96:128, 96:96 + NB])

                # softmax scratch + attn staging + WaH live from after the
                # transition weights free up (stack-ordered pools)
                dec2p = scn.enter_context(tc.tile_pool(name="dec2", bufs=1))
                exp_sc = dec2p.tile([128, 256], f32)
                align_bf = dec2p.tile([128, 256], bf)
                dve_t = dec2p.tile([128, 256], bf)
                attnT = dec2p.tile([128, 8, 64], bf)   # per-chunk staging
                wah_sb = dec2p.tile([128, 8, U], bf)
                # wah rides the idle gpsimd queue; needed first at t=15
                for kk in range(8):
                    nc.gpsimd.dma_start(out=wah_sb[:, kk, :],
                                        in_=WaH_t[128 * kk:128 * (kk + 1), :])

                def softmax_emit(t):
                    # scores in psum_sc rows 96:100 -> alTall[:, :, t, :]
                    nc.scalar.activation(exp_sc[96:128, :], psum_sc[96:128, :], AF.Exp)
                    for b in range(NB):
                        nc.vector.reduce_sum(rsums[96:128, b:b + 1],
                                             exp_sc[96:128, 64 * b:64 * (b + 1)],
                                             axis=AX.X)
                    nc.vector.tensor_mul(rsums[96:128, :], rsums[96:128, :],
                                         rmask[96:128, :])
                    nc.vector.reduce_sum(rsD[96:128, :], rsums[96:128, :], axis=AX.X)
                    nc.vector.reciprocal(rsD[96:128, :], rsD[96:128, :])
                    nc.vector.tensor_scalar(align_bf[96:128, :], exp_sc[96:128, :],
                                            rsD[96:128, 0:1], None, op0=ALU.mult)
                    nc.vector.transpose(dve_t[96:128, :], align_bf[96:128, :])
                    # diag value align_b[32h+r] sits at dve_t[96+r, 32*(2b+h)+b]
                    for b in range(NB):
                        p, q = b // 2, b % 2
                        for hh in range(2):
                            cc = 32 * (2 * b + hh) + b
                            nc.vector.tensor_copy(
                                alTall[64 * q + 32 * hh:64 * q + 32 * hh + 32,
                                       p, t, b:b + 1],
                                dve_t[96:128, cc:cc + 1])

                def attn_chunk(j):
                    c0, c1 = CHUNKS[j]
                    cw = (c1 - c0) * NB
                    for ko in range(8):
                        pa = psum_mw[:, 0:cw]
                        for kk in range(8):
                            nc.tensor.matmul(
                                pa, wah_sb[:, kk, 128 * ko:128 * (ko + 1)],
                                HdecT[:, kk, 1 + c0:1 + c1, :],
                                start=(kk == 0), stop=False)
                        for p in range(2):
                            nc.tensor.matmul(
                                pa,
                                MemWaC[:, p, 128 * ko:128 * (ko + 1)],
                                alTall[:, p, c0:c1, :].rearrange(
                                    "p t b -> p (t b)"),
                                start=False, stop=(p == 1))
                        nc.vector.tensor_copy(attnT[:, ko, 0:cw], pa)
                    nc.gpsimd.dma_start(
                        out=aginC[j][:].rearrange("k p c -> p k c"),
                        in_=attnT[:, :, 0:cw])
                    nc.gpsimd.collective_compute(
                        "AllGather", ALU.bypass,
                        ins=[aginC[j][:]], outs=[agoutC[j][:]],
                        replica_groups=[list(range(NC))])

                # streamed t=0 weights (Whd0) in a scoped pool
                with ExitStack() as c4:
                    w0p = c4.enter_context(tc.tile_pool(name="w0", bufs=4))
                    w0_tiles = []
                    for kk in range(8):
                        w0 = w0p.tile([128, G4], bf, tag="w0")
                        nc.gpsimd.dma_start(out=w0[:],
                                            in_=Whd0[128 * kk:128 * (kk + 1), :])
                        w0_tiles.append(w0)

                    nc.sync.dma_start(out=xe_pp[0][:], in_=Xd_d[0:NB, :])
                    ps = psum_zp[0]
                    nc.sync.dma_start(out=xe_pp[1][:], in_=Xd_d[NB:2 * NB, :])
                    # kk_outer so the streamed w0 ping-pong never waits on a
                    # later-pc matmul (deadlock)
                    emit_z_stream(ps, xe_pp[0],
                                  (lambda kk: memT[:, kk, TS - 1, :]),
                                  (lambda kk: w0_tiles[kk]),
                                  first=False, with_align=False, kk_outer=True)
                    gate_tail(ps, HdecT[:, :, 1, :])
                    for kk in range(8):
                        nc.tensor.matmul(
                            psum_sc[96:96 + NB, :],
                            HdecT[:, kk, 1, :],
                            keysT[:, kk].rearrange("p b s -> p (b s)"),
                            start=(kk == 0), stop=(kk == 7),
                            tile_position=(0, 96))
                    softmax_emit(0)

                for t in range(1, TD_RUN):
                    xd = xe_pp[t % 2]
                    ps = psum_zp[t % 2]
                    if t + 1 < TD_RUN:
                        nc.sync.dma_start(out=xe_pp[(t + 1) % 2][:],
                                          in_=Xd_d[NB * (t + 1):NB * (t + 2), :])
                    emit_z_stream(ps, xd,
                                  (lambda kk, _t=t: HdecT[:, kk, _t, :]),
                                  (lambda kk: whc_sb[:, kk, :]),
                                  first=False, with_align=True, al_t=t - 1)
                    gate_tail(ps, HdecT[:, :, t + 1, :])
                    for kk in range(8):
                        nc.tensor.matmul(
                            psum_sc[96:96 + NB, :],
                            HdecT[:, kk, t + 1, :],
                            keysT[:, kk].rearrange("p b s -> p (b s)"),
                            start=(kk == 0), stop=(kk == 7),
                            tile_position=(0, 96))
                    softmax_emit(t)
                    if stage == "full" and (t + 1) in [c1 for _, c1 in CHUNKS]:
                        attn_chunk([c1 for _, c1 in CHUNKS].index(t + 1))

                if debug:
                    nc.sync.dma_start(out=dbg["HallT"][:], in_=HdecT[:])
                    nc.sync.dma_start(out=dbg["alTall"][:], in_=alTall[:])

        # ------- projection (activations staged from the AllGather DRAM bufs) ---
        if stage == "full":
            with ExitStack() as c2:
                ppd = c2.enter_context(tc.tile_pool(name="projd", bufs=3))
                ps4 = c2.enter_context(tc.tile_pool(name="projps", bufs=8, space="PSUM"))
                stp = c2.enter_context(tc.tile_pool(name="projag", bufs=3))
                NCH = VSH // 500
                wfp = c2.enter_context(tc.tile_pool(name="wfc", bufs=2))
                for sc in range(NCH):
                    wf_c = wfp.tile([128, 8, 500], bf, tag="wfc")
                    for kk in range(8):
                        nc.scalar.dma_start(
                            out=wf_c[:, kk, :],
                            in_=Wfs[128 * kk:128 * (kk + 1), 500 * sc:500 * (sc + 1)])
                    bfc = wfp.tile([128, 500], f32, tag="bfc")
                    nc.scalar.dma_start(
                        out=bfc[:],
                        in_=bfs[:, 500 * sc:500 * (sc + 1)].to_broadcast([128, 500]))
                    for r in range(NC):
                        for th in range(2):
                            t0 = 32 * th
                            t1 = min(t0 + 32, TD)
                            rr = (t1 - t0) * NB
                            r0 = 252 * r + NB * t0
                            ag = stp.tile([128, 8, 32, NB], bf, tag="ag")
                            for j, (c0, c1) in enumerate(CHUNKS):
                                ov0, ov1 = max(c0, t0), min(c1, t1)
                                if ov0 >= ov1:
                                    continue
                                nc.gpsimd.dma_start(
                                    out=ag[:, :, ov0 - t0:ov1 - t0, :],
                                    in_=agoutC[j][r].rearrange(
                                        "k p (t b) -> p k t b",
                                        b=NB)[:, :, ov0 - c0:ov1 - c0, :])
                            pj = ps4.tile([128, 500], f32, tag="pj")
                            for kk in range(8):
                                nc.tensor.matmul(
                                    pj[:rr, :],
                                    ag[:, kk, 0:t1 - t0, :].rearrange(
                                        "p t b -> p (t b)"),
                                    wf_c[:, kk, :],
                                    start=(kk == 0), stop=(kk == 7))
                            st = ppd.tile([128, 500], f32, tag="st")
                            nc.vector.tensor_add(st[:rr, :], pj[:rr, :], bfc[:rr, :])
                            nc.sync.dma_start(
                                out=logits[r0:r0 + rr, 500 * sc:500 * (sc + 1)],
                                in_=st[:rr, :])
        else:
            # partial-stage dummy output so the NEFF has its ExternalOutput written
            st0 = gp.tile([1, 4], f32, tag="dummy")
            nc.vector.tensor_copy(st0[:], tga[0:1, 0:4])
            nc.sync.dma_start(out=logits[0:1, 0:4], in_=st0[:])

    nc.finalize()
    return nc, dbg


_CACHE = {}


def _get_nc(stage="full", debug=False):
    key = (stage, debug)
    if key not in _CACHE:
        _CACHE[key] = _build_nc(stage, debug)
    return _CACHE[key]


def run_cores(inputs, stage="full", debug=False, trace=False):
    from concourse.bass_utils import run_bass_kernel_spmd
    shared, per_core = _prep_host(inputs)
    nc, dbg = _get_nc(stage, debug)
    in_maps = []
    for k in range(NC):
        m = dict(shared)
        m.update(per_core[k])
        in_maps.append(m)
    return run_bass_kernel_spmd(nc, in_maps, core_ids=list(range(NC)), trace=trace)


def unshard(outs):
    full = np.concatenate(outs, axis=1)                     # [2016, 32000]
    # rows ordered (r, t, b_local); batch b = 4*r + b_local
    full = full.reshape(NC, TD, NB, VT).transpose(0, 2, 1, 3).reshape(B, TD, VT)
    return np.ascontiguousarray(full.astype(np.float32))


def kernel(**inputs):
    res = run_cores(inputs, stage="full")
    outs = [np.asarray(r["logits"]) for r in res.results]   # [2016, 4000] each
    return unshard(outs)


# revision 26
# speedup vs baseline: 1.2875x; 1.2222x over previous
"""Trainium2 Bass kernel for nn_DmTranslateTrain (seq2seq translate train step).

Strategy (8 NeuronCores, SPMD):
  - Data-parallel over batch: core k owns batches [4k, 4k+4). Each core runs the
    full encoder LSTM scan + decoder (LSTM + Luong attention) for its 4 batches.
  - Output projection is tensor-parallel over the vocabulary: chunked AllGather
    of attention activations overlapped with the decoder, then each core
    computes logits[:, 4000k:4000k+4000].

Scan-step design (the hot loop):
  - The x-projection (emb @ Wx + b, precomputed in DRAM) is folded into the PE
    accumulation with a tiny K=4 identity matmul, so the gate nonlinearities
    read PSUM directly (no vector adds on the critical path).
  - All four gates use plain tanh: sigma(x) = (1+tanh(x/2))/2, with the 0.5
    pre-scale for gates i/f/o folded into the weight columns host-side.  One
    fused tanh per 512-col gate pair (2 ACTs per step), one activation table.
  - State kept scaled: C2 = 2c, H = 2h.  Updates via scalar_tensor_tensor:
      IG2 = (ti+1)*tu; FC2 = (tf+1)*C2; C2' = 0.5*FC2 + IG2;
      tc = tanh(0.5*C2'); H = (to+1)*tc.
  - z matmuls emitted kk-outer / m-inner so the 4 PE column-groups
    (tile_position=(0,32m)) stream concurrently; decoder scores run in column
    group q96 on separate PSUM partitions.

Gate packing: z tile is [128, 1024] per band m (partition = 32*m + b), free
col = gate*256 + 32*fc + r for unit u = 128*fc + 32*m + r, gates ordered
[u, i, f, o] (u = candidate).  The DVE 32x32 block transpose of the H tile
directly yields H^T in natural u-major chunks (one copy per step).
Logits rows are ordered (core, t, local batch); the host unshards.
"""

import numpy as np

B, TS, TD = 32, 64, 63
VS, VT = 32000, 32000
E, U = 256, 1024
G4 = 4 * U
NB = 4            # batches per core
NC = 8            # cores
VSH = VT // NC    # vocab shard per core
RE = TS * NB      # encoder rows per core
RD = TD * NB      # decoder rows per core
RT = TD * B       # total decoder rows (all batches)

_GATE_PERM = [2, 0, 1, 3]  # new order [u, i, f, o] -> original gate index
CHUNKS = [(0, 16), (16, 32), (32, 48), (48, 60), (60, TD)]


def _reorder_cols(w):
    # natural col = gate_orig*1024 + u, u = 128*fc + 32*m + r
    w5 = w.reshape(w.shape[0], 4, 8, 4, 32)        # [in, g_orig, fc, m, r]
    w5 = w5[:, _GATE_PERM]                          # [in, g_new, fc, m, r]
    w5 = w5.transpose(0, 3, 1, 2, 4)                # [in, m, g_new, fc, r]
    return np.ascontiguousarray(w5.reshape(w.shape[0], G4))


def _reorder_bias(b):
    b5 = b.reshape(4, 8, 4, 32)[_GATE_PERM].transpose(2, 0, 1, 3)
    return np.ascontiguousarray(b5.reshape(1, G4))


def _prep_host(inputs):
    import ml_dtypes
    bf16 = ml_dtypes.bfloat16
    f32 = np.float32
    enc_in = np.asarray(inputs["encoder_input"])
    dec_in = np.asarray(inputs["decoder_input"])
    Wx_e = np.asarray(inputs["Wx_e"], f32)
    Wh_e = np.asarray(inputs["Wh_e"], f32)
    b_e = np.asarray(inputs["b_e"], f32)
    Wx_d = np.asarray(inputs["Wx_d"], f32)
    Wh_d = np.asarray(inputs["Wh_d"], f32)
    b_d = np.asarray(inputs["b_d"], f32)
    Wm = np.asarray(inputs["Wm"], f32)
    Wa = np.asarray(inputs["Wa"], f32)
    Wf = np.asarray(inputs["Wf"], f32)
    bfv = np.asarray(inputs["bf"], f32)

    Wxd_x = Wx_d[:E]
    Wxd_a = Wx_d[E:]
    Wa_h, Wa_c = Wa[:U], Wa[U:]

    # per-gate column scale on the NATURAL layout (i, f, g, o): tanh trick
    # needs 0.5*z for i/f/o; the candidate gate g keeps full scale.
    cs = np.concatenate([np.full(U, 0.5, f32), np.full(U, 0.5, f32),
                         np.ones(U, f32), np.full(U, 0.5, f32)])

    shared = {
        "Wxe": _reorder_cols(Wx_e * cs).astype(bf16),
        "Whe": _reorder_cols(0.5 * Wh_e * cs).astype(bf16),
        "Whcomb": _reorder_cols(0.5 * (Wh_d + Wa_h @ Wxd_a) * cs).astype(bf16),
        "Wca": _reorder_cols(0.5 * (Wa_c @ Wxd_a) * cs).astype(bf16),
        "Whd0": _reorder_cols(0.5 * Wh_d * cs).astype(bf16),
        "Wxdx": _reorder_cols(Wxd_x * cs).astype(bf16),
        "Wm": (0.25 * Wm).astype(bf16),
        "WaH": (0.5 * Wa_h).astype(bf16),
        "WaC": np.ascontiguousarray((0.5 * Wa_c).astype(bf16)),
        "be": _reorder_bias(b_e * cs),
        "bd": _reorder_bias(b_d * cs),
        "enc_emb": np.ascontiguousarray(np.asarray(inputs["enc_emb"], f32)),
        "dec_emb": np.ascontiguousarray(np.asarray(inputs["dec_emb"], f32)),
    }
    Wf_bf = Wf.astype(bf16)
    per_core = []
    for k in range(NC):
        eidx = enc_in[NB * k:NB * (k + 1)]
        didx = dec_in[NB * k:NB * (k + 1)]
        per_core.append({
            "enc_idx": np.ascontiguousarray(eidx.T.reshape(RE, 1).astype(np.int32)),
            "dec_idx": np.ascontiguousarray(didx.T.reshape(RD, 1).astype(np.int32)),
            "Wfs": np.ascontiguousarray(Wf_bf[:, VSH * k:VSH * (k + 1)]),
            "bfs": np.ascontiguousarray(bfv[VSH * k:VSH * (k + 1)].reshape(1, VSH)),
        })
    return shared, per_core


# ---------------------------------------------------------------------------

def _build_nc(stage="full", debug=False):
    import re as _re
    from contextlib import ExitStack
    import concourse.bass as bass
    import concourse.mybir as mybir
    import concourse.tile as tile
    from concourse import bacc
    from concourse.masks import make_identity

    dt = mybir.dt
    AF = mybir.ActivationFunctionType
    ALU = mybir.AluOpType
    AX = mybir.AxisListType
    f32, bf = dt.float32, dt.bfloat16

    nc = bacc.Bacc("TRN2", target_bir_lowering=False, debug=False, num_devices=NC)

    enc_idx = nc.dram_tensor("enc_idx", [RE, 1], dt.int32, kind="ExternalInput")
    dec_idx = nc.dram_tensor("dec_idx", [RD, 1], dt.int32, kind="ExternalInput")
    enc_emb = nc.dram_tensor("enc_emb", [VS, E], f32, kind="ExternalInput")
    dec_emb = nc.dram_tensor("dec_emb", [VT, E], f32, kind="ExternalInput")
    Wxe = nc.dram_tensor("Wxe", [E, G4], bf, kind="ExternalInput")
    Whe = nc.dram_tensor("Whe", [U, G4], bf, kind="ExternalInput")
    Whcomb = nc.dram_tensor("Whcomb", [U, G4], bf, kind="ExternalInput")
    Wca_t = nc.dram_tensor("Wca", [U, G4], bf, kind="ExternalInput")
    Whd0 = nc.dram_tensor("Whd0", [U, G4], bf, kind="ExternalInput")
    Wxdx = nc.dram_tensor("Wxdx", [E, G4], bf, kind="ExternalInput")
    Wm_t = nc.dram_tensor("Wm", [U, U], bf, kind="ExternalInput")
    WaH_t = nc.dram_tensor("WaH", [U, U], bf, kind="ExternalInput")
    WaC_t = nc.dram_tensor("WaC", [U, U], bf, kind="ExternalInput")
    Wfs = nc.dram_tensor("Wfs", [U, VSH], bf, kind="ExternalInput")
    bfs = nc.dram_tensor("bfs", [1, VSH], f32, kind="ExternalInput")
    be_t = nc.dram_tensor("be", [1, G4], f32, kind="ExternalInput")
    bd_t = nc.dram_tensor("bd", [1, G4], f32, kind="ExternalInput")

    logits = nc.dram_tensor("logits", [RT, VSH], f32, kind="ExternalOutput")

    dbg = {}
    if debug:
        dbg["memT"] = nc.dram_tensor("dbg_memT", [128, 8, TS, NB], bf, kind="ExternalOutput")
        dbg["c_enc"] = nc.dram_tensor("dbg_cenc", [128, 256], f32, kind="ExternalOutput")
        dbg["keysT"] = nc.dram_tensor("dbg_keysT", [128, 8, NB, TS], bf, kind="ExternalOutput")
        dbg["HallT"] = nc.dram_tensor("dbg_HallT", [128, 8, TD + 1, NB], bf, kind="ExternalOutput")
        dbg["alTall"] = nc.dram_tensor("dbg_alTall", [128, 2, TD, NB], bf, kind="ExternalOutput")
        dbg["MemWca"] = nc.dram_tensor("dbg_MemWca", [128, 2, G4], bf, kind="ExternalOutput")

    with tile.TileContext(nc) as tc, ExitStack() as ctx:
        constp = ctx.enter_context(tc.tile_pool(name="const", bufs=1))
        ident = constp.tile([128, 128], bf)
        make_identity(nc, ident[:])

        dramp = ctx.enter_context(tc.tile_pool(name="dram", bufs=1, space="DRAM"))
        Xe_d = dramp.tile([RE, G4], bf, tag="Xe")
        Xd_d = dramp.tile([RD, G4], bf, tag="Xd")
        aginC = [dramp.tile([8, 128, (c1 - c0) * NB], bf, tag=f"agin{j}",
                            name=f"aginC{j}")
                 for j, (c0, c1) in enumerate(CHUNKS)]
        agoutC = [dramp.tile([NC, 8, 128, (c1 - c0) * NB], bf, tag=f"agout{j}",
                             name=f"agoutC{j}", addr_space="Shared")
                  for j, (c0, c1) in enumerate(CHUNKS)]

        statep = ctx.enter_context(tc.tile_pool(name="state", bufs=1))
        memT = statep.tile([128, 8, TS, NB], bf)       # encoder H^T (= 2h)
        C2 = statep.tile([128, 256], f32)              # 2c (enc then dec)
        keysT = statep.tile([128, 8, NB, TS], bf)      # keys^T, batch-major
        HdecT = statep.tile([128, 8, TD + 1, NB], bf)  # slot t+1 = H_t = 2h_t
        alTall = statep.tile([128, 2, TD, NB], bf)     # block-diag align rows=(q,s), cols=b
        MemWca = statep.tile([128, 2, G4], bf)         # (memT @ Wca'), rows=(q,s)
        MemWaC = statep.tile([128, 2, U], bf)          # (memT @ WaC'), rows=(q,s)

        gp = ctx.enter_context(tc.tile_pool(name="gates", bufs=1))
        xe_pp = [gp.tile([NB, G4], bf, name=f"xe{i}") for i in range(2)]
        tga = gp.tile([128, 512], f32)   # tanh(z_u), tanh(z_i/2)
        tfo = gp.tile([128, 512], f32)   # tanh(z_f/2), tanh(z_o/2)
        IG2 = gp.tile([128, 256], f32)
        FC2 = gp.tile([128, 256], f32)
        tc_t = gp.tile([128, 256], f32)
        Hbf = gp.tile([128, 256], bf)
        h_tr = gp.tile([128, 256], bf, tag="h_tr")

        # ------------- embedding gathers + X precomputes -------------
        def x_precompute_all(jobs):
            with ExitStack() as c2:
                pp = c2.enter_context(tc.tile_pool(name="xpre", bufs=2))
                pp1 = c2.enter_context(tc.tile_pool(name="xpre1", bufs=1))
                psx = c2.enter_context(tc.tile_pool(name="xpre_ps", bufs=1, space="PSUM"))
                tiles = []
                for jj, (idx_t, emb_t, w_t, bias_t, rows, out_d) in enumerate(jobs):
                    nm = (rows + 127) // 128
                    for m in range(nm):
                        r0 = 128 * m
                        rr = min(128 * (m + 1), rows) - r0
                        idx_sb = pp1.tile([128, 1], dt.int32, name=f"idx{jj}_{m}")
                        nc.sync.dma_start(out=idx_sb[:rr, :], in_=idx_t[r0:r0 + rr, :])
                        gath = pp1.tile([128, E], f32, name=f"gath{jj}_{m}")
                        nc.gpsimd.indirect_dma_start(
                            out=gath[:rr, :], out_offset=None,
                            in_=emb_t[:],
                            in_offset=bass.IndirectOffsetOnAxis(ap=idx_sb[:rr, :1],
                                                                axis=0))
                        gbf = pp1.tile([128, E], bf, name=f"gbf{jj}_{m}")
                        nc.vector.tensor_copy(gbf[:rr, :], gath[:rr, :])
                        tiles.append((jj, r0, rr, gbf))
                w_sb = pp1.tile([128, 2, G4], bf, name="wx")
                bias_bc = pp1.tile([128, G4], f32, name="biasbc")
                cur = [None]

                def _stage_wb(jj):
                    w_t, bias_t = jobs[jj][2], jobs[jj][3]
                    for kk in range(2):
                        nc.scalar.dma_start(out=w_sb[:, kk, :],
                                            in_=w_t[128 * kk:128 * (kk + 1), :])
                    nc.scalar.dma_start(out=bias_bc[:],
                                        in_=bias_t[:].to_broadcast([128, G4]))
                    cur[0] = jj

                for jj, r0, rr, gbf in tiles:
                    if cur[0] != jj:
                        _stage_wb(jj)
                    out_d = jobs[jj][5]
                    xT = pp.tile([128, 2, 128], bf, tag="xT")
                    for kk in range(2):
                        pt = psx.tile([128, 128], bf, tag="ptr")
                        nc.tensor.transpose(pt[:, :rr], gbf[:rr, 128 * kk:128 * (kk + 1)],
                                            ident[:rr, :rr])
                        nc.vector.tensor_copy(xT[:, kk, :rr], pt[:, :rr])
                    for chv in range(8):
                        cs0 = 512 * chv
                        ps = psx.tile([128, 512], f32, tag="pmm")
                        for kk in range(2):
                            nc.tensor.matmul(ps[:rr, :], xT[:, kk, :rr],
                                             w_sb[:, kk, cs0:cs0 + 512],
                                             start=(kk == 0), stop=(kk == 1))
                        st = pp.tile([128, 512], bf, tag="stage")
                        nc.vector.tensor_add(st[:rr, :], ps[:rr, :],
                                             bias_bc[:rr, cs0:cs0 + 512])
                        nc.sync.dma_start(out=out_d[r0:r0 + rr, cs0:cs0 + 512],
                                          in_=st[:rr, :])

        def gate_tail(ps, dst_of_h):
            # z in psum ps [128, 1024]; writes H^T into dst_of_h(h) [128, 4, NB]
            # for kk half h, updates C2 in place.  Split into fc-halves so the
            # first half of H^T (kk 0..3) lands early and the next z-stream
            # restarts sooner.
            ps4 = ps[:].rearrange("p (g c) -> p g c", g=4)
            tga4 = tga[:].rearrange("p (g c) -> p g c", g=2)
            tfo4 = tfo[:].rearrange("p (g c) -> p g c", g=2)
            for h in range(2):
                cl, ch = 128 * h, 128 * h + 128
                nc.scalar.activation(tga4[:, :, cl:ch], ps4[:, 0:2, cl:ch],
                                     AF.Tanh)
                nc.scalar.activation(tfo4[:, :, cl:ch], ps4[:, 2:4, cl:ch],
                                     AF.Tanh)
                nc.vector.scalar_tensor_tensor(
                    IG2[:, cl:ch], tga[:, 256 + cl:256 + ch], 1.0,
                    tga[:, cl:ch], op0=ALU.add, op1=ALU.mult)
                nc.vector.scalar_tensor_tensor(
                    FC2[:, cl:ch], tfo[:, cl:ch], 1.0,
                    C2[:, cl:ch], op0=ALU.add, op1=ALU.mult)
                nc.vector.scalar_tensor_tensor(
                    C2[:, cl:ch], FC2[:, cl:ch], 0.5,
                    IG2[:, cl:ch], op0=ALU.mult, op1=ALU.add)
                nc.scalar.activation(tc_t[:, cl:ch], C2[:, cl:ch],
                                     AF.Tanh, scale=0.5)
                nc.vector.scalar_tensor_tensor(
                    Hbf[:, cl:ch], tfo[:, 256 + cl:256 + ch], 1.0,
                    tc_t[:, cl:ch], op0=ALU.add, op1=ALU.mult)
                nc.vector.transpose(h_tr[:, cl:ch], Hbf[:, cl:ch])
                nc.vector.tensor_copy(
                    dst_of_h(h),
                    h_tr[:, cl:ch].rearrange("p (k c) -> p k c", k=4)[:, :, 0:NB])

        # ------------- scans (shared psum pool) -------------
        with ExitStack() as scn:
            psp = scn.enter_context(tc.tile_pool(name="scanps", bufs=1, space="PSUM"))
            psum_z0 = psp.tile([128, 1024], f32, tag="pz0")
            psum_z1 = psp.tile([128, 1024], f32, tag="pz1")
            psum_zp = [psum_z0, psum_z1]
            psum_sc = psp.tile([128, 256], f32, tag="psc")
            psum_mw = psp.tile([128, 512], f32, tag="pmw")

            def emit_ids(ps, xe, close):
                # identity matmuls fold the x projection into psum (group start)
                for m in range(4):
                    for chv in range(2):
                        co = 1024 * m + 512 * chv
                        nc.tensor.matmul(
                            ps[32 * m:32 * m + NB, 512 * chv:512 * chv + 512],
                            ident[0:NB, 0:NB], xe[0:NB, co:co + 512],
                            start=True, stop=close,
                            tile_position=(0, 32 * m))

            def emit_z_stream(ps, lhsT_of_kk, w_sb_of_kk, with_align,
                              al_t=None):
                # kk-outer, chv-inner: per col group the two chv matmuls share
                # one stationary load (bass skips the redundant LDWEIGHTS)
                for kk in range(8):
                    lh = lhsT_of_kk(kk)
                    for m in range(4):
                        for chv in range(2):
                            co = 1024 * m + 512 * chv
                            nc.tensor.matmul(
                                ps[32 * m:32 * m + NB, 512 * chv:512 * chv + 512],
                                lh, w_sb_of_kk(kk)[:, co:co + 512],
                                start=False,
                                stop=(kk == 7 and not with_align),
                                tile_position=(0, 32 * m))
                if with_align:
                    for m in range(4):
                        for p in range(2):
                            for chv in range(2):
                                co = 1024 * m + 512 * chv
                                nc.tensor.matmul(
                                    ps[32 * m:32 * m + NB, 512 * chv:512 * chv + 512],
                                    alTall[:, p, al_t, :],
                                    MemWca[:, p, co:co + 512],
                                    start=False, stop=(p == 1),
                                    tile_position=(0, 32 * m))

            # x precompute first: its staging pools need the space the big
            # weight pools occupy later.
            x_precompute_all([
                (enc_idx, enc_emb, Wxe, be_t, RE, Xe_d),
                (dec_idx, dec_emb, Wxdx, bd_t, RD, Xd_d),
            ])

            # Whcomb is prefetched during the encoder (gpsimd queue is idle);
            # its pool lives for the whole scan scope.
            whcp = scn.enter_context(tc.tile_pool(name="whc", bufs=1))
            whc_sb = whcp.tile([128, 8, G4], bf)

            # ---------------- encoder ----------------
            with ExitStack() as ec:
                encp = ec.enter_context(tc.tile_pool(name="enc", bufs=1))
                whe_sb = encp.tile([128, 8, G4], bf)
                for kk in range(8):
                    nc.scalar.dma_start(out=whe_sb[:, kk, :],
                                        in_=Whe[128 * kk:128 * (kk + 1), :])

                nc.vector.memset(C2[:], 0.0)

                nc.sync.dma_start(out=xe_pp[0][:], in_=Xe_d[0:NB, :])
                emit_ids(psum_zp[0], xe_pp[0], close=True)
                for t in range(TS):
                    ps = psum_zp[t % 2]
                    if t + 1 < TS:
                        nc.sync.dma_start(out=xe_pp[(t + 1) % 2][:],
                                          in_=Xe_d[NB * (t + 1):NB * (t + 2), :])
                    if t > 0:
                        emit_z_stream(ps,
                                      (lambda kk, _t=t: memT[:, kk, _t - 1, :]),
                                      (lambda kk: whe_sb[:, kk, :]),
                                      with_align=False)
                    # next step's id matmuls go in front of the tail so they
                    # fill the PE gap (they only need the x tile)
                    if t + 1 < TS:
                        emit_ids(psum_zp[(t + 1) % 2], xe_pp[(t + 1) % 2],
                                 close=False)
                    gate_tail(ps, (lambda h, _t=t:
                                   memT[:, 4 * h:4 * h + 4, _t, :]))
                    # prefetch Whcomb on the idle gpsimd queue
                    if stage != "enc" and t == 40:
                        for kk in range(8):
                            nc.gpsimd.dma_start(
                                out=whc_sb[:, kk, :],
                                in_=Whcomb[128 * kk:128 * (kk + 1), :])

                if debug:
                    nc.sync.dma_start(out=dbg["memT"][:], in_=memT[:])
                    nc.sync.dma_start(out=dbg["c_enc"][:], in_=C2[:])

            # ---------------- transition: keys, MemWca, MemWaC ----------------
            m_dec = _re.match(r"dec(\d+)$", stage)
            TD_RUN = int(m_dec.group(1)) if m_dec else TD
            if stage != "enc":
                decp = scn.enter_context(tc.tile_pool(name="dec", bufs=1))

                memQ = decp.tile([128, 8, 2, 128], bf)

                with ExitStack() as c3:
                    wmp = c3.enter_context(tc.tile_pool(name="wmp", bufs=1))
                    wm_sb = wmp.tile([128, 8, U], bf)
                    for kk in range(8):
                        nc.gpsimd.dma_start(out=wm_sb[:, kk, :],
                                            in_=Wm_t[128 * kk:128 * (kk + 1), :])
                    # keysT = (memT @ Wm')^T, stored batch-major [p, kk, b, s]
                    for ko in range(8):
                        for kk in range(8):
                            nc.tensor.matmul(psum_mw[:, 0:256],
                                             wm_sb[:, kk, 128 * ko:128 * (ko + 1)],
                                             memT[:, kk, :, :],
                                             start=(kk == 0), stop=(kk == 7))
                        nc.vector.tensor_copy(
                            keysT[:, ko],
                            psum_mw[:, 0:256].rearrange("p (s b) -> p b s", b=NB))

                    # memQ[:, kk, p, 64q+s] = memT[:, kk, s, 2p+q]
                    for kk in range(8):
                        for p in range(2):
                            nc.vector.tensor_copy(
                                memQ[:, kk, p, :].rearrange("p (q s) -> p q s", q=2),
                                memT[:, kk, :, 2 * p:2 * p + 2].rearrange(
                                    "p s q -> p q s"))

                with ExitStack() as c3b:
                    wcap2 = c3b.enter_context(tc.tile_pool(name="wca2", bufs=1))
                    wca_sb = wcap2.tile([128, 8, G4], bf)
                    for kk in range(8):
                        nc.gpsimd.dma_start(out=wca_sb[:, kk, :],
                                            in_=Wca_t[128 * kk:128 * (kk + 1), :])
                    for p in range(2):
                        for c8 in range(8):
                            for kk in range(8):
                                nc.tensor.matmul(
                                    psum_mw[:], memQ[:, kk, p, :],
                                    wca_sb[:, kk, 512 * c8:512 * (c8 + 1)],
                                    start=(kk == 0), stop=(kk == 7))
                            nc.vector.tensor_copy(
                                MemWca[:, p, 512 * c8:512 * (c8 + 1)], psum_mw[:])

                with ExitStack() as c3c:
                    wacp = c3c.enter_context(tc.tile_pool(name="wacp", bufs=1))
                    wac_sb = wacp.tile([128, 8, U], bf)
                    for kk in range(8):
                        nc.gpsimd.dma_start(out=wac_sb[:, kk, :],
                                            in_=WaC_t[128 * kk:128 * (kk + 1), :])
                    for p in range(2):
                        for c2_ in range(2):
                            for kk in range(8):
                                nc.tensor.matmul(
                                    psum_mw[:], memQ[:, kk, p, :],
                                    wac_sb[:, kk, 512 * c2_:512 * (c2_ + 1)],
                                    start=(kk == 0), stop=(kk == 7))
                            nc.vector.tensor_copy(
                                MemWaC[:, p, 512 * c2_:512 * (c2_ + 1)], psum_mw[:])

                if debug:
                    nc.sync.dma_start(out=dbg["keysT"][:], in_=keysT[:])
                    nc.sync.dma_start(out=dbg["MemWca"][:], in_=MemWca[:])

                # ---------------- decoder scan ----------------
                nc.vector.memset(alTall[:], 0.0)

                rsums = decp.tile([128, NB], f32)
                rmask = decp.tile([128, NB], f32)
                rsD = decp.tile([128, 1], f32)
                # rmask[96+p, b] = 1 iff p == b (diag selector)
                nc.vector.tensor_copy(rmask[96:128, :], ident[96:128, 96:96 + NB])

                exp_sc = None
                align_bf = None
                dve_t = None
                attnT = None
                wah_sb = None
                sb_ag = None

                def softmax_emit(t):
                    # scores in psum_sc rows 96:100 -> alTall[:, :, t, :]
                    nc.scalar.activation(exp_sc[96:128, :], psum_sc[96:128, :], AF.Exp)
                    for b in range(NB):
                        nc.vector.reduce_sum(rsums[96:128, b:b + 1],
                                             exp_sc[96:128, 64 * b:64 * (b + 1)],
                                             axis=AX.X)
                    nc.vector.tensor_mul(rsums[96:128, :], rsums[96:128, :],
                                         rmask[96:128, :])
                    nc.vector.reduce_sum(rsD[96:128, :], rsums[96:128, :], axis=AX.X)
                    nc.vector.reciprocal(rsD[96:128, :], rsD[96:128, :])
                    nc.vector.tensor_scalar(align_bf[96:128, :], exp_sc[96:128, :],
                                            rsD[96:128, 0:1], None, op0=ALU.mult)
                    nc.vector.transpose(dve_t[96:128, :], align_bf[96:128, :])
                    # diag value align_b[32h+r] sits at dve_t[96+r, 32*(2b+h)+b]
                    for b in range(NB):
                        p, q = b // 2, b % 2
                        for hh in range(2):
                            cc = 32 * (2 * b + hh) + b
                            nc.vector.tensor_copy(
                                alTall[64 * q + 32 * hh:64 * q + 32 * hh + 32,
                                       p, t, b:b + 1],
                                dve_t[96:128, cc:cc + 1])

                def attn_chunk(j):
                    c0, c1 = CHUNKS[j]
                    cw = (c1 - c0) * NB
                    for ko in range(8):
                        pa = psum_mw[:, 0:cw]
                        for kk in range(8):
                            nc.tensor.matmul(
                                pa, wah_sb[:, kk, 128 * ko:128 * (ko + 1)],
                                HdecT[:, kk, 1 + c0:1 + c1, :],
                                start=(kk == 0), stop=False)
                        for p in range(2):
                            nc.tensor.matmul(
                                pa,
                                MemWaC[:, p, 128 * ko:128 * (ko + 1)],
                                alTall[:, p, c0:c1, :].rearrange(
                                    "p t b -> p (t b)"),
                                start=False, stop=(p == 1))
                        nc.vector.tensor_copy(attnT[:, ko, 0:cw], pa)
                    nc.gpsimd.dma_start(
                        out=aginC[j][:].rearrange("k p c -> p k c"),
                        in_=attnT[:, :, 0:cw])
                    nc.gpsimd.collective_compute(
                        "AllGather", ALU.bypass,
                        ins=[aginC[j][:]], outs=[agoutC[j][:]],
                        replica_groups=[list(range(NC))])
                    for r in range(NC):
                        nc.gpsimd.dma_start(
                            out=sb_ag[:, r, :, c0:c1, :],
                            in_=agoutC[j][r].rearrange("k p (t b) -> p k t b",
                                                       b=NB))

                def scores_emit(t):
                    for kk in range(8):
                        nc.tensor.matmul(
                            psum_sc[96:96 + NB, :],
                            HdecT[:, kk, t + 1, :],
                            keysT[:, kk].rearrange("p b s -> p (b s)"),
                            start=(kk == 0), stop=(kk == 7),
                            tile_position=(0, 96))

                # streamed t=0 weights (Whd0) in a scoped pool
                with ExitStack() as c4:
                    w0p = c4.enter_context(tc.tile_pool(name="w0", bufs=4))
                    w0_tiles = []
                    for kk in range(8):
                        w0 = w0p.tile([128, G4], bf, tag="w0")
                        nc.gpsimd.dma_start(out=w0[:],
                                            in_=Whd0[128 * kk:128 * (kk + 1), :])
                        w0_tiles.append(w0)

                    nc.sync.dma_start(out=xe_pp[0][:], in_=Xd_d[0:NB, :])
                    ps = psum_zp[0]
                    nc.sync.dma_start(out=xe_pp[1][:], in_=Xd_d[NB:2 * NB, :])
                    emit_ids(ps, xe_pp[0], close=False)
                    emit_z_stream(ps,
                                  (lambda kk: memT[:, kk, TS - 1, :]),
                                  (lambda kk: w0_tiles[kk]),
                                  with_align=False)
                    emit_ids(psum_zp[1], xe_pp[1], close=False)
                    gate_tail(ps, (lambda h: HdecT[:, 4 * h:4 * h + 4, 1, :]))
                    scores_emit(0)

                # softmax scratch + attn staging + WaH + gathered activations
                # (allocated after the w0 pool frees its space)
                dec2p = scn.enter_context(tc.tile_pool(name="dec2", bufs=1))
                exp_sc = dec2p.tile([128, 256], f32)
                align_bf = dec2p.tile([128, 256], bf)
                dve_t = dec2p.tile([128, 256], bf)
                attnT = dec2p.tile([128, 8, 64], bf)   # per-chunk staging
                wah_sb = dec2p.tile([128, 8, U], bf)
                sb_ag = dec2p.tile([128, NC, 8, TD, NB], bf)
                # wah rides the idle gpsimd queue; needed first at t=15
                for kk in range(8):
                    nc.gpsimd.dma_start(out=wah_sb[:, kk, :],
                                        in_=WaH_t[128 * kk:128 * (kk + 1), :])
                softmax_emit(0)

                for t in range(1, TD_RUN):
                    ps = psum_zp[t % 2]
                    if t + 1 < TD_RUN:
                        nc.sync.dma_start(out=xe_pp[(t + 1) % 2][:],
                                          in_=Xd_d[NB * (t + 1):NB * (t + 2), :])
                    emit_z_stream(ps,
                                  (lambda kk, _t=t: HdecT[:, kk, _t, :]),
                                  (lambda kk: whc_sb[:, kk, :]),
                                  with_align=True, al_t=t - 1)
                    if t + 1 < TD_RUN:
                        emit_ids(psum_zp[(t + 1) % 2], xe_pp[(t + 1) % 2],
                                 close=False)
                    gate_tail(ps, (lambda h, _t=t:
                                   HdecT[:, 4 * h:4 * h + 4, _t + 1, :]))
                    scores_emit(t)
                    softmax_emit(t)
                    if stage == "full" and (t + 1) in [c1 for _, c1 in CHUNKS]:
                        attn_chunk([c1 for _, c1 in CHUNKS].index(t + 1))

                if debug:
                    nc.sync.dma_start(out=dbg["HallT"][:], in_=HdecT[:])
                    nc.sync.dma_start(out=dbg["alTall"][:], in_=alTall[:])

            # ------- projection (sb_ag filled by the chunked AllGather) -------
            if stage == "full":
                with ExitStack() as c2:
                    ppd = c2.enter_context(tc.tile_pool(name="projd", bufs=2))
                    ps4 = c2.enter_context(tc.tile_pool(name="projps", bufs=2,
                                                        space="PSUM"))
                    NCH = VSH // 500
                    wfp = c2.enter_context(tc.tile_pool(name="wfc", bufs=2))
                    for sc in range(NCH):
                        wf_c = wfp.tile([128, 8, 500], bf, tag="wfc")
                        for kk in range(8):
                            nc.scalar.dma_start(
                                out=wf_c[:, kk, :],
                                in_=Wfs[128 * kk:128 * (kk + 1),
                                        500 * sc:500 * (sc + 1)])
                        bfc = wfp.tile([128, 500], f32, tag="bfc")
                        nc.scalar.dma_start(
                            out=bfc[:],
                            in_=bfs[:, 500 * sc:500 * (sc + 1)].to_broadcast(
                                [128, 500]))
                        for r in range(NC):
                            for th in range(2):
                                t0 = 32 * th
                                t1 = min(t0 + 32, TD)
                                rr = (t1 - t0) * NB
                                r0 = 252 * r + NB * t0
                                pj = ps4.tile([128, 500], f32, tag="pj")
                                for kk in range(8):
                                    nc.tensor.matmul(
                                        pj[:rr, :],
                                        sb_ag[:, r, kk, t0:t1, :].rearrange(
                                            "p t b -> p (t b)"),
                                        wf_c[:, kk, :],
                                        start=(kk == 0), stop=(kk == 7))
                                st = ppd.tile([128, 500], f32, tag="st")
                                nc.vector.tensor_add(st[:rr, :], pj[:rr, :],
                                                     bfc[:rr, :])
                                nc.sync.dma_start(
                                    out=logits[r0:r0 + rr,
                                               500 * sc:500 * (sc + 1)],
                                    in_=st[:rr, :])

        if stage != "full":
            # partial-stage dummy output so the NEFF has its ExternalOutput written
            st0 = gp.tile([1, 4], f32, tag="dummy")
            nc.vector.tensor_copy(st0[:], tga[0:1, 0:4])
            nc.sync.dma_start(out=logits[0:1, 0:4], in_=st0[:])

    nc.finalize()
    return nc, dbg


_CACHE = {}


def _get_nc(stage="full", debug=False):
    key = (stage, debug)
    if key not in _CACHE:
        _CACHE[key] = _build_nc(stage, debug)
    return _CACHE[key]


def run_cores(inputs, stage="full", debug=False, trace=False):
    from concourse.bass_utils import run_bass_kernel_spmd
    shared, per_core = _prep_host(inputs)
    nc, dbg = _get_nc(stage, debug)
    in_maps = []
    for k in range(NC):
        m = dict(shared)
        m.update(per_core[k])
        in_maps.append(m)
    return run_bass_kernel_spmd(nc, in_maps, core_ids=list(range(NC)), trace=trace)


def unshard(outs):
    full = np.concatenate(outs, axis=1)                     # [2016, 32000]
    # rows ordered (r, t, b_local); batch b = 4*r + b_local
    full = full.reshape(NC, TD, NB, VT).transpose(0, 2, 1, 3).reshape(B, TD, VT)
    return np.ascontiguousarray(full.astype(np.float32))


def kernel(**inputs):
    res = run_cores(inputs, stage="full")
    outs = [np.asarray(r["logits"]) for r in res.results]   # [2016, 4000] each
    return unshard(outs)


# revision 32
# speedup vs baseline: 1.3532x; 1.0510x over previous
"""Trainium2 Bass kernel for nn_DmTranslateTrain (seq2seq translate train step).

Strategy (8 NeuronCores, SPMD):
  - Data-parallel over batch: core k owns batches [4k, 4k+4). Each core runs the
    full encoder LSTM scan + decoder (LSTM + Luong attention) for its 4 batches.
  - Output projection is tensor-parallel over the vocabulary: chunked AllGather
    of attention activations overlapped with the decoder, then each core
    computes logits[:, 4000k:4000k+4000].

Scan-step design (the hot loop):
  - The x-projection (emb @ Wx + b, precomputed in DRAM) is folded into the PE
    accumulation with a tiny K=4 identity matmul, so the gate nonlinearities
    read PSUM directly (no vector adds on the critical path).
  - All four gates use plain tanh: sigma(x) = (1+tanh(x/2))/2, with the 0.5
    pre-scale for gates i/f/o folded into the weight columns host-side.  One
    fused tanh per 512-col gate pair (2 ACTs per step), one activation table.
  - State kept scaled: C2 = 2c, H = 2h.  Updates via scalar_tensor_tensor:
      IG2 = (ti+1)*tu; FC2 = (tf+1)*C2; C2' = 0.5*FC2 + IG2;
      tc = tanh(0.5*C2'); H = (to+1)*tc.
  - z matmuls emitted kk-outer / m-inner so the 4 PE column-groups
    (tile_position=(0,32m)) stream concurrently; decoder scores run in column
    group q96 on separate PSUM partitions.

Gate packing: z tile is [128, 1024] per band m (partition = 32*m + b), free
col = gate*256 + 32*fc + r for unit u = 128*fc + 32*m + r, gates ordered
[u, i, f, o] (u = candidate).  The DVE 32x32 block transpose of the H tile
directly yields H^T in natural u-major chunks (one copy per step).
Logits rows are ordered (core, t, local batch); the host unshards.
"""

import numpy as np

B, TS, TD = 32, 64, 63
VS, VT = 32000, 32000
E, U = 256, 1024
G4 = 4 * U
NB = 4            # batches per core
NC = 8            # cores
VSH = VT // NC    # vocab shard per core
RE = TS * NB      # encoder rows per core
RD = TD * NB      # decoder rows per core
RT = TD * B       # total decoder rows (all batches)

_GATE_PERM = [2, 0, 1, 3]  # new order [u, i, f, o] -> original gate index
CHUNKS = [(0, 16), (16, 32), (32, 48), (48, 60), (60, TD)]


def _reorder_cols(w):
    # natural col = gate_orig*1024 + u, u = 128*fc + 32*m + r
    w5 = w.reshape(w.shape[0], 4, 8, 4, 32)        # [in, g_orig, fc, m, r]
    w5 = w5[:, _GATE_PERM]                          # [in, g_new, fc, m, r]
    w5 = w5.transpose(0, 3, 1, 2, 4)                # [in, m, g_new, fc, r]
    return np.ascontiguousarray(w5.reshape(w.shape[0], G4))


def _reorder_bias(b):
    b5 = b.reshape(4, 8, 4, 32)[_GATE_PERM].transpose(2, 0, 1, 3)
    return np.ascontiguousarray(b5.reshape(1, G4))


def _prep_host(inputs):
    import ml_dtypes
    bf16 = ml_dtypes.bfloat16
    f32 = np.float32
    enc_in = np.asarray(inputs["encoder_input"])
    dec_in = np.asarray(inputs["decoder_input"])
    Wx_e = np.asarray(inputs["Wx_e"], f32)
    Wh_e = np.asarray(inputs["Wh_e"], f32)
    b_e = np.asarray(inputs["b_e"], f32)
    Wx_d = np.asarray(inputs["Wx_d"], f32)
    Wh_d = np.asarray(inputs["Wh_d"], f32)
    b_d = np.asarray(inputs["b_d"], f32)
    Wm = np.asarray(inputs["Wm"], f32)
    Wa = np.asarray(inputs["Wa"], f32)
    Wf = np.asarray(inputs["Wf"], f32)
    bfv = np.asarray(inputs["bf"], f32)

    Wxd_x = Wx_d[:E]
    Wxd_a = Wx_d[E:]
    Wa_h, Wa_c = Wa[:U], Wa[U:]

    # per-gate column scale on the NATURAL layout (i, f, g, o): tanh trick
    # needs 0.5*z for i/f/o; the candidate gate g keeps full scale.
    cs = np.concatenate([np.full(U, 0.5, f32), np.full(U, 0.5, f32),
                         np.ones(U, f32), np.full(U, 0.5, f32)])

    shared = {
        "Wxe": _reorder_cols(Wx_e * cs).astype(bf16),
        "Whe": _reorder_cols(0.5 * Wh_e * cs).astype(bf16),
        "Whcomb": _reorder_cols(0.5 * (Wh_d + Wa_h @ Wxd_a) * cs).astype(bf16),
        "Wca": _reorder_cols(0.5 * (Wa_c @ Wxd_a) * cs).astype(bf16),
        "Whd0": _reorder_cols(0.5 * Wh_d * cs).astype(bf16),
        "Wxdx": _reorder_cols(Wxd_x * cs).astype(bf16),
        "Wm": (0.25 * Wm).astype(bf16),
        "WaH": (0.5 * Wa_h).astype(bf16),
        "WaC": np.ascontiguousarray((0.5 * Wa_c).astype(bf16)),
        "be": _reorder_bias(b_e * cs),
        "bd": _reorder_bias(b_d * cs),
        "enc_emb": np.ascontiguousarray(np.asarray(inputs["enc_emb"], f32)),
        "dec_emb": np.ascontiguousarray(np.asarray(inputs["dec_emb"], f32)),
    }
    Wf_bf = Wf.astype(bf16)
    per_core = []
    for k in range(NC):
        eidx = enc_in[NB * k:NB * (k + 1)]
        didx = dec_in[NB * k:NB * (k + 1)]
        per_core.append({
            "enc_idx": np.ascontiguousarray(eidx.T.reshape(RE, 1).astype(np.int32)),
            "dec_idx": np.ascontiguousarray(didx.T.reshape(RD, 1).astype(np.int32)),
            "Wfs": np.ascontiguousarray(Wf_bf[:, VSH * k:VSH * (k + 1)]),
            "bfs": np.ascontiguousarray(bfv[VSH * k:VSH * (k + 1)].reshape(1, VSH)),
        })
    return shared, per_core


# ---------------------------------------------------------------------------

def _build_nc(stage="full", debug=False):
    import re as _re
    from contextlib import ExitStack
    import concourse.bass as bass
    import concourse.mybir as mybir
    import concourse.tile as tile
    from concourse import bacc
    from concourse.masks import make_identity

    dt = mybir.dt
    AF = mybir.ActivationFunctionType
    ALU = mybir.AluOpType
    AX = mybir.AxisListType
    f32, bf = dt.float32, dt.bfloat16

    nc = bacc.Bacc("TRN2", target_bir_lowering=False, debug=False, num_devices=NC)

    enc_idx = nc.dram_tensor("enc_idx", [RE, 1], dt.int32, kind="ExternalInput")
    dec_idx = nc.dram_tensor("dec_idx", [RD, 1], dt.int32, kind="ExternalInput")
    enc_emb = nc.dram_tensor("enc_emb", [VS, E], f32, kind="ExternalInput")
    dec_emb = nc.dram_tensor("dec_emb", [VT, E], f32, kind="ExternalInput")
    Wxe = nc.dram_tensor("Wxe", [E, G4], bf, kind="ExternalInput")
    Whe = nc.dram_tensor("Whe", [U, G4], bf, kind="ExternalInput")
    Whcomb = nc.dram_tensor("Whcomb", [U, G4], bf, kind="ExternalInput")
    Wca_t = nc.dram_tensor("Wca", [U, G4], bf, kind="ExternalInput")
    Whd0 = nc.dram_tensor("Whd0", [U, G4], bf, kind="ExternalInput")
    Wxdx = nc.dram_tensor("Wxdx", [E, G4], bf, kind="ExternalInput")
    Wm_t = nc.dram_tensor("Wm", [U, U], bf, kind="ExternalInput")
    WaH_t = nc.dram_tensor("WaH", [U, U], bf, kind="ExternalInput")
    WaC_t = nc.dram_tensor("WaC", [U, U], bf, kind="ExternalInput")
    Wfs = nc.dram_tensor("Wfs", [U, VSH], bf, kind="ExternalInput")
    bfs = nc.dram_tensor("bfs", [1, VSH], f32, kind="ExternalInput")
    be_t = nc.dram_tensor("be", [1, G4], f32, kind="ExternalInput")
    bd_t = nc.dram_tensor("bd", [1, G4], f32, kind="ExternalInput")

    logits = nc.dram_tensor("logits", [RT, VSH], f32, kind="ExternalOutput")

    dbg = {}
    if debug:
        dbg["memT"] = nc.dram_tensor("dbg_memT", [128, 8, TS, NB], bf, kind="ExternalOutput")
        dbg["c_enc"] = nc.dram_tensor("dbg_cenc", [128, 256], f32, kind="ExternalOutput")
        dbg["keysT"] = nc.dram_tensor("dbg_keysT", [128, 8, NB, TS], bf, kind="ExternalOutput")
        dbg["HallT"] = nc.dram_tensor("dbg_HallT", [128, 8, TD + 1, NB], bf, kind="ExternalOutput")
        dbg["alTall"] = nc.dram_tensor("dbg_alTall", [128, 2, TD, NB], bf, kind="ExternalOutput")
        dbg["MemWca"] = nc.dram_tensor("dbg_MemWca", [128, 2, G4], bf, kind="ExternalOutput")

    with tile.TileContext(nc) as tc, ExitStack() as ctx:
        constp = ctx.enter_context(tc.tile_pool(name="const", bufs=1))
        ident = constp.tile([128, 128], bf)
        make_identity(nc, ident[:])

        dramp = ctx.enter_context(tc.tile_pool(name="dram", bufs=1, space="DRAM"))
        Xe_d = dramp.tile([RE, G4], bf, tag="Xe")
        Xd_d = dramp.tile([RD, G4], bf, tag="Xd")
        aginC = [dramp.tile([8, 128, (c1 - c0) * NB], bf, tag=f"agin{j}",
                            name=f"aginC{j}")
                 for j, (c0, c1) in enumerate(CHUNKS)]
        agoutC = [dramp.tile([NC, 8, 128, (c1 - c0) * NB], bf, tag=f"agout{j}",
                             name=f"agoutC{j}", addr_space="Shared")
                  for j, (c0, c1) in enumerate(CHUNKS)]

        statep = ctx.enter_context(tc.tile_pool(name="state", bufs=1))
        memT = statep.tile([128, 8, TS, NB], bf)       # encoder H^T (= 2h)
        C2 = statep.tile([128, 256], f32)              # 2c (enc then dec)
        keysT = statep.tile([128, 8, NB, TS], bf)      # keys^T, batch-major
        HdecT = statep.tile([128, 8, TD + 1, NB], bf)  # slot t+1 = H_t = 2h_t
        alTall = statep.tile([128, 2, TD, NB], bf)     # block-diag align rows=(q,s), cols=b
        MemWca = statep.tile([128, 2, G4], bf)         # (memT @ Wca'), rows=(q,s)
        MemWaC = statep.tile([128, 2, U], bf)          # (memT @ WaC'), rows=(q,s)

        gp = ctx.enter_context(tc.tile_pool(name="gates", bufs=1))
        xe_pp = [gp.tile([NB, G4], bf, name=f"xe{i}") for i in range(2)]
        tga = gp.tile([128, 512], f32)   # tanh(z_u), tanh(z_i/2)
        tfo = gp.tile([128, 512], f32)   # tanh(z_f/2), tanh(z_o/2)
        IG2 = gp.tile([128, 256], f32)
        FC2 = gp.tile([128, 256], f32)
        tc_t = gp.tile([128, 256], f32)
        Hbf = gp.tile([128, 256], bf)
        h_tr = gp.tile([128, 256], bf, tag="h_tr")

        # ------------- embedding gathers + X precomputes -------------
        def x_precompute_all(jobs):
            with ExitStack() as c2:
                pp = c2.enter_context(tc.tile_pool(name="xpre", bufs=2))
                pp1 = c2.enter_context(tc.tile_pool(name="xpre1", bufs=1))
                psx = c2.enter_context(tc.tile_pool(name="xpre_ps", bufs=1, space="PSUM"))
                tiles = []
                for jj, (idx_t, emb_t, w_t, bias_t, rows, out_d) in enumerate(jobs):
                    nm = (rows + 127) // 128
                    for m in range(nm):
                        r0 = 128 * m
                        rr = min(128 * (m + 1), rows) - r0
                        idx_sb = pp1.tile([128, 1], dt.int32, name=f"idx{jj}_{m}")
                        nc.sync.dma_start(out=idx_sb[:rr, :], in_=idx_t[r0:r0 + rr, :])
                        gath = pp1.tile([128, E], f32, name=f"gath{jj}_{m}")
                        nc.gpsimd.indirect_dma_start(
                            out=gath[:rr, :], out_offset=None,
                            in_=emb_t[:],
                            in_offset=bass.IndirectOffsetOnAxis(ap=idx_sb[:rr, :1],
                                                                axis=0))
                        gbf = pp1.tile([128, E], bf, name=f"gbf{jj}_{m}")
                        nc.vector.tensor_copy(gbf[:rr, :], gath[:rr, :])
                        tiles.append((jj, r0, rr, gbf))
                w_sb = pp1.tile([128, 2, G4], bf, name="wx")
                bias_bc = pp1.tile([128, G4], f32, name="biasbc")
                cur = [None]

                def _stage_wb(jj):
                    w_t, bias_t = jobs[jj][2], jobs[jj][3]
                    for kk in range(2):
                        nc.scalar.dma_start(out=w_sb[:, kk, :],
                                            in_=w_t[128 * kk:128 * (kk + 1), :])
                    nc.scalar.dma_start(out=bias_bc[:],
                                        in_=bias_t[:].to_broadcast([128, G4]))
                    cur[0] = jj

                for jj, r0, rr, gbf in tiles:
                    if cur[0] != jj:
                        _stage_wb(jj)
                    out_d = jobs[jj][5]
                    xT = pp.tile([128, 2, 128], bf, tag="xT")
                    for kk in range(2):
                        pt = psx.tile([128, 128], bf, tag="ptr")
                        nc.tensor.transpose(pt[:, :rr], gbf[:rr, 128 * kk:128 * (kk + 1)],
                                            ident[:rr, :rr])
                        nc.vector.tensor_copy(xT[:, kk, :rr], pt[:, :rr])
                    for chv in range(8):
                        cs0 = 512 * chv
                        ps = psx.tile([128, 512], f32, tag="pmm")
                        for kk in range(2):
                            nc.tensor.matmul(ps[:rr, :], xT[:, kk, :rr],
                                             w_sb[:, kk, cs0:cs0 + 512],
                                             start=(kk == 0), stop=(kk == 1))
                        st = pp.tile([128, 512], bf, tag="stage")
                        nc.vector.tensor_add(st[:rr, :], ps[:rr, :],
                                             bias_bc[:rr, cs0:cs0 + 512])
                        nc.sync.dma_start(out=out_d[r0:r0 + rr, cs0:cs0 + 512],
                                          in_=st[:rr, :])

        def gate_tail(ps, dst_of_h):
            # z in psum ps [128, 1024]; writes H^T into dst_of_h(h) [128, 4, NB]
            # for kk half h, updates C2 in place.  Split into fc-halves so the
            # first half of H^T (kk 0..3) lands early and the next z-stream
            # restarts sooner.
            ps4 = ps[:].rearrange("p (g c) -> p g c", g=4)
            tga4 = tga[:].rearrange("p (g c) -> p g c", g=2)
            tfo4 = tfo[:].rearrange("p (g c) -> p g c", g=2)
            for h in range(2):
                cl, ch = 128 * h, 128 * h + 128
                nc.scalar.activation(tga4[:, :, cl:ch], ps4[:, 0:2, cl:ch],
                                     AF.Tanh)
                nc.scalar.activation(tfo4[:, :, cl:ch], ps4[:, 2:4, cl:ch],
                                     AF.Tanh)
                nc.vector.scalar_tensor_tensor(
                    IG2[:, cl:ch], tga[:, 256 + cl:256 + ch], 1.0,
                    tga[:, cl:ch], op0=ALU.add, op1=ALU.mult)
                nc.vector.scalar_tensor_tensor(
                    FC2[:, cl:ch], tfo[:, cl:ch], 1.0,
                    C2[:, cl:ch], op0=ALU.add, op1=ALU.mult)
                nc.vector.scalar_tensor_tensor(
                    C2[:, cl:ch], FC2[:, cl:ch], 0.5,
                    IG2[:, cl:ch], op0=ALU.mult, op1=ALU.add)
                nc.scalar.activation(tc_t[:, cl:ch], C2[:, cl:ch],
                                     AF.Tanh, scale=0.5)
                nc.vector.scalar_tensor_tensor(
                    Hbf[:, cl:ch], tfo[:, 256 + cl:256 + ch], 1.0,
                    tc_t[:, cl:ch], op0=ALU.add, op1=ALU.mult)
                nc.vector.transpose(h_tr[:, cl:ch], Hbf[:, cl:ch])
                nc.vector.tensor_copy(
                    dst_of_h(h),
                    h_tr[:, cl:ch].rearrange("p (k c) -> p k c", k=4)[:, :, 0:NB])

        # gathered attention activations: scattered per AllGather chunk during
        # the decoder, consumed by the projection after the scan scope closes.
        # Must sit below the scan pools in the pool stack.
        sbagp = ctx.enter_context(tc.tile_pool(name="sbag", bufs=1))
        sb_ag = sbagp.tile([128, NC, 8, TD, NB], bf)

        # ------------- scans (shared psum pool) -------------
        with ExitStack() as scn:
            psp = scn.enter_context(tc.tile_pool(name="scanps", bufs=1, space="PSUM"))
            psum_z0 = psp.tile([128, 1024], f32, tag="pz0")
            psum_z1 = psp.tile([128, 1024], f32, tag="pz1")
            psum_zp = [psum_z0, psum_z1]
            psum_sc = psp.tile([128, 256], f32, tag="psc")
            psum_mw = psp.tile([128, 512], f32, tag="pmw")

            def emit_ids(ps, xe, close):
                # identity matmuls fold the x projection into psum (group start)
                for m in range(4):
                    for chv in range(2):
                        co = 1024 * m + 512 * chv
                        nc.tensor.matmul(
                            ps[32 * m:32 * m + NB, 512 * chv:512 * chv + 512],
                            ident[0:NB, 0:NB], xe[0:NB, co:co + 512],
                            start=True, stop=close,
                            tile_position=(0, 32 * m))

            def emit_z_stream(ps, lhsT_of_kk, w_sb_of_kk, with_align,
                              al_t=None):
                # kk-outer, chv-inner: per col group the two chv matmuls share
                # one stationary load (bass skips the redundant LDWEIGHTS)
                for kk in range(8):
                    lh = lhsT_of_kk(kk)
                    for m in range(4):
                        for chv in range(2):
                            co = 1024 * m + 512 * chv
                            nc.tensor.matmul(
                                ps[32 * m:32 * m + NB, 512 * chv:512 * chv + 512],
                                lh, w_sb_of_kk(kk)[:, co:co + 512],
                                start=False,
                                stop=(kk == 7 and not with_align),
                                tile_position=(0, 32 * m))
                if with_align:
                    for m in range(4):
                        for p in range(2):
                            for chv in range(2):
                                co = 1024 * m + 512 * chv
                                nc.tensor.matmul(
                                    ps[32 * m:32 * m + NB, 512 * chv:512 * chv + 512],
                                    alTall[:, p, al_t, :],
                                    MemWca[:, p, co:co + 512],
                                    start=False, stop=(p == 1),
                                    tile_position=(0, 32 * m))

            # x precompute first: its staging pools need the space the big
            # weight pools occupy later.
            x_precompute_all([
                (enc_idx, enc_emb, Wxe, be_t, RE, Xe_d),
                (dec_idx, dec_emb, Wxdx, bd_t, RD, Xd_d),
            ])

            # Whcomb: 6 chunks prefetched during the encoder (gpsimd queue
            # is idle); the last 2 chunks load once Wca's space frees up.
            whcp = scn.enter_context(tc.tile_pool(name="whc", bufs=1))
            whc_a = whcp.tile([128, 6, G4], bf)
            whc_b = None

            def whc_of_kk(kk):
                return whc_a[:, kk, :] if kk < 6 else whc_b[:, kk - 6, :]

            # ---------------- encoder ----------------
            with ExitStack() as ec:
                encp = ec.enter_context(tc.tile_pool(name="enc", bufs=1))
                whe_sb = encp.tile([128, 8, G4], bf)
                for kk in range(8):
                    nc.scalar.dma_start(out=whe_sb[:, kk, :],
                                        in_=Whe[128 * kk:128 * (kk + 1), :])

                nc.vector.memset(C2[:], 0.0)

                nc.sync.dma_start(out=xe_pp[0][:], in_=Xe_d[0:NB, :])
                emit_ids(psum_zp[0], xe_pp[0], close=True)
                for t in range(TS):
                    ps = psum_zp[t % 2]
                    if t + 1 < TS:
                        nc.sync.dma_start(out=xe_pp[(t + 1) % 2][:],
                                          in_=Xe_d[NB * (t + 1):NB * (t + 2), :])
                    if t > 0:
                        emit_z_stream(ps,
                                      (lambda kk, _t=t: memT[:, kk, _t - 1, :]),
                                      (lambda kk: whe_sb[:, kk, :]),
                                      with_align=False)
                    # next step's id matmuls go in front of the tail so they
                    # fill the PE gap (they only need the x tile)
                    if t + 1 < TS:
                        emit_ids(psum_zp[(t + 1) % 2], xe_pp[(t + 1) % 2],
                                 close=False)
                    gate_tail(ps, (lambda h, _t=t:
                                   memT[:, 4 * h:4 * h + 4, _t, :]))
                    # prefetch most of Whcomb on the idle gpsimd queue
                    # (last 2 chunks wait for the Wca space at the transition)
                    if stage != "enc" and t == 40:
                        for kk in range(6):
                            nc.gpsimd.dma_start(
                                out=whc_a[:, kk, :],
                                in_=Whcomb[128 * kk:128 * (kk + 1), :])

                if debug:
                    nc.sync.dma_start(out=dbg["memT"][:], in_=memT[:])
                    nc.sync.dma_start(out=dbg["c_enc"][:], in_=C2[:])

            # ---------------- transition: keys, MemWca, MemWaC ----------------
            m_dec = _re.match(r"dec(\d+)$", stage)
            TD_RUN = int(m_dec.group(1)) if m_dec else TD
            if stage != "enc":
                decp = scn.enter_context(tc.tile_pool(name="dec", bufs=1))

                memQ = decp.tile([128, 8, 2, 128], bf)

                with ExitStack() as c3:
                    wmp = c3.enter_context(tc.tile_pool(name="wmp", bufs=1))
                    wm_sb = wmp.tile([128, 8, U], bf)
                    for kk in range(8):
                        nc.gpsimd.dma_start(out=wm_sb[:, kk, :],
                                            in_=Wm_t[128 * kk:128 * (kk + 1), :])
                    # keysT = (memT @ Wm')^T, stored batch-major [p, kk, b, s]
                    for ko in range(8):
                        for kk in range(8):
                            nc.tensor.matmul(psum_mw[:, 0:256],
                                             wm_sb[:, kk, 128 * ko:128 * (ko + 1)],
                                             memT[:, kk, :, :],
                                             start=(kk == 0), stop=(kk == 7))
                        nc.vector.tensor_copy(
                            keysT[:, ko],
                            psum_mw[:, 0:256].rearrange("p (s b) -> p b s", b=NB))

                    # memQ[:, kk, p, 64q+s] = memT[:, kk, s, 2p+q]
                    for kk in range(8):
                        for p in range(2):
                            nc.vector.tensor_copy(
                                memQ[:, kk, p, :].rearrange("p (q s) -> p q s", q=2),
                                memT[:, kk, :, 2 * p:2 * p + 2].rearrange(
                                    "p s q -> p q s"))

                with ExitStack() as c3b:
                    wcap2 = c3b.enter_context(tc.tile_pool(name="wca2", bufs=1))
                    wca_sb = wcap2.tile([128, 8, G4], bf)
                    for kk in range(8):
                        nc.gpsimd.dma_start(out=wca_sb[:, kk, :],
                                            in_=Wca_t[128 * kk:128 * (kk + 1), :])
                    for p in range(2):
                        for c8 in range(8):
                            for kk in range(8):
                                nc.tensor.matmul(
                                    psum_mw[:], memQ[:, kk, p, :],
                                    wca_sb[:, kk, 512 * c8:512 * (c8 + 1)],
                                    start=(kk == 0), stop=(kk == 7))
                            nc.vector.tensor_copy(
                                MemWca[:, p, 512 * c8:512 * (c8 + 1)], psum_mw[:])

                whcp2 = scn.enter_context(tc.tile_pool(name="whc2", bufs=1))
                whc_b = whcp2.tile([128, 2, G4], bf)
                for kk in range(6, 8):
                    nc.gpsimd.dma_start(out=whc_b[:, kk - 6, :],
                                        in_=Whcomb[128 * kk:128 * (kk + 1), :])
                # decoder x tiles can load as soon as the encoder stops
                # touching the ping-pong buffers
                nc.sync.dma_start(out=xe_pp[0][:], in_=Xd_d[0:NB, :])
                nc.sync.dma_start(out=xe_pp[1][:], in_=Xd_d[NB:2 * NB, :])

                with ExitStack() as c3c:
                    wacp = c3c.enter_context(tc.tile_pool(name="wacp", bufs=1))
                    wac_sb = wacp.tile([128, 8, U], bf)
                    for kk in range(8):
                        nc.gpsimd.dma_start(out=wac_sb[:, kk, :],
                                            in_=WaC_t[128 * kk:128 * (kk + 1), :])
                    for p in range(2):
                        for c2_ in range(2):
                            for kk in range(8):
                                nc.tensor.matmul(
                                    psum_mw[:], memQ[:, kk, p, :],
                                    wac_sb[:, kk, 512 * c2_:512 * (c2_ + 1)],
                                    start=(kk == 0), stop=(kk == 7))
                            nc.vector.tensor_copy(
                                MemWaC[:, p, 512 * c2_:512 * (c2_ + 1)], psum_mw[:])

                if debug:
                    nc.sync.dma_start(out=dbg["keysT"][:], in_=keysT[:])
                    nc.sync.dma_start(out=dbg["MemWca"][:], in_=MemWca[:])

                # ---------------- decoder scan ----------------
                nc.vector.memset(alTall[:], 0.0)

                rsums = decp.tile([128, NB], f32)
                rmask = decp.tile([128, NB], f32)
                rsD = decp.tile([128, 1], f32)
                # rmask[96+p, b] = 1 iff p == b (diag selector)
                nc.vector.tensor_copy(rmask[96:128, :], ident[96:128, 96:96 + NB])

                exp_sc = None
                align_bf = None
                dve_t = None
                attnT = None
                wah_sb = None

                def softmax_emit(t):
                    # scores in psum_sc rows 96:100 -> alTall[:, :, t, :]
                    nc.scalar.activation(exp_sc[96:128, :], psum_sc[96:128, :], AF.Exp)
                    for b in range(NB):
                        nc.vector.reduce_sum(rsums[96:128, b:b + 1],
                                             exp_sc[96:128, 64 * b:64 * (b + 1)],
                                             axis=AX.X)
                    nc.vector.tensor_mul(rsums[96:128, :], rsums[96:128, :],
                                         rmask[96:128, :])
                    nc.vector.reduce_sum(rsD[96:128, :], rsums[96:128, :], axis=AX.X)
                    nc.vector.reciprocal(rsD[96:128, :], rsD[96:128, :])
                    nc.vector.tensor_scalar(align_bf[96:128, :], exp_sc[96:128, :],
                                            rsD[96:128, 0:1], None, op0=ALU.mult)
                    nc.vector.transpose(dve_t[96:128, :], align_bf[96:128, :])
                    # diag value align_b[32h+r] sits at dve_t[96+r, 32*(2b+h)+b]
                    for b in range(NB):
                        p, q = b // 2, b % 2
                        for hh in range(2):
                            cc = 32 * (2 * b + hh) + b
                            nc.vector.tensor_copy(
                                alTall[64 * q + 32 * hh:64 * q + 32 * hh + 32,
                                       p, t, b:b + 1],
                                dve_t[96:128, cc:cc + 1])

                def attn_chunk(j):
                    c0, c1 = CHUNKS[j]
                    cw = (c1 - c0) * NB
                    for ko in range(8):
                        pa = psum_mw[:, 0:cw]
                        for kk in range(8):
                            nc.tensor.matmul(
                                pa, wah_sb[:, kk, 128 * ko:128 * (ko + 1)],
                                HdecT[:, kk, 1 + c0:1 + c1, :],
                                start=(kk == 0), stop=False)
                        for p in range(2):
                            nc.tensor.matmul(
                                pa,
                                MemWaC[:, p, 128 * ko:128 * (ko + 1)],
                                alTall[:, p, c0:c1, :].rearrange(
                                    "p t b -> p (t b)"),
                                start=False, stop=(p == 1))
                        nc.vector.tensor_copy(attnT[:, ko, 0:cw], pa)
                    nc.gpsimd.dma_start(
                        out=aginC[j][:].rearrange("k p c -> p k c"),
                        in_=attnT[:, :, 0:cw])
                    nc.gpsimd.collective_compute(
                        "AllGather", ALU.bypass,
                        ins=[aginC[j][:]], outs=[agoutC[j][:]],
                        replica_groups=[list(range(NC))])
                    for r in range(NC):
                        nc.gpsimd.dma_start(
                            out=sb_ag[:, r, :, c0:c1, :],
                            in_=agoutC[j][r].rearrange("k p (t b) -> p k t b",
                                                       b=NB))

                def scores_emit(t):
                    for kk in range(8):
                        nc.tensor.matmul(
                            psum_sc[96:96 + NB, :],
                            HdecT[:, kk, t + 1, :],
                            keysT[:, kk].rearrange("p b s -> p (b s)"),
                            start=(kk == 0), stop=(kk == 7),
                            tile_position=(0, 96))

                # streamed t=0 weights (Whd0) in a scoped pool
                with ExitStack() as c4:
                    w0p = c4.enter_context(tc.tile_pool(name="w0", bufs=3))
                    w0_tiles = []
                    for kk in range(8):
                        w0 = w0p.tile([128, G4], bf, tag="w0")
                        nc.gpsimd.dma_start(out=w0[:],
                                            in_=Whd0[128 * kk:128 * (kk + 1), :])
                        w0_tiles.append(w0)

                    ps = psum_zp[0]
                    emit_ids(ps, xe_pp[0], close=False)
                    emit_z_stream(ps,
                                  (lambda kk: memT[:, kk, TS - 1, :]),
                                  (lambda kk: w0_tiles[kk]),
                                  with_align=False)
                    emit_ids(psum_zp[1], xe_pp[1], close=False)
                    gate_tail(ps, (lambda h: HdecT[:, 4 * h:4 * h + 4, 1, :]))
                    scores_emit(0)

                # softmax scratch + attn staging + WaH + gathered activations
                # (allocated after the w0 pool frees its space)
                dec2p = scn.enter_context(tc.tile_pool(name="dec2", bufs=1))
                exp_sc = dec2p.tile([128, 256], f32)
                align_bf = dec2p.tile([128, 256], bf)
                dve_t = dec2p.tile([128, 256], bf)
                attnT = dec2p.tile([128, 8, 64], bf)   # per-chunk staging
                wah_sb = dec2p.tile([128, 8, U], bf)
                # wah rides the idle gpsimd queue; needed first at t=15
                for kk in range(8):
                    nc.gpsimd.dma_start(out=wah_sb[:, kk, :],
                                        in_=WaH_t[128 * kk:128 * (kk + 1), :])
                softmax_emit(0)

                for t in range(1, TD_RUN):
                    ps = psum_zp[t % 2]
                    if t + 1 < TD_RUN:
                        nc.sync.dma_start(out=xe_pp[(t + 1) % 2][:],
                                          in_=Xd_d[NB * (t + 1):NB * (t + 2), :])
                    emit_z_stream(ps,
                                  (lambda kk, _t=t: HdecT[:, kk, _t, :]),
                                  whc_of_kk,
                                  with_align=True, al_t=t - 1)
                    if t + 1 < TD_RUN:
                        emit_ids(psum_zp[(t + 1) % 2], xe_pp[(t + 1) % 2],
                                 close=False)
                    gate_tail(ps, (lambda h, _t=t:
                                   HdecT[:, 4 * h:4 * h + 4, _t + 1, :]))
                    scores_emit(t)
                    softmax_emit(t)
                    if stage == "full" and (t + 1) in [c1 for _, c1 in CHUNKS]:
                        attn_chunk([c1 for _, c1 in CHUNKS].index(t + 1))

                if debug:
                    nc.sync.dma_start(out=dbg["HallT"][:], in_=HdecT[:])
                    nc.sync.dma_start(out=dbg["alTall"][:], in_=alTall[:])

        # ------- projection (sb_ag filled by the chunked AllGather) -------
        if stage == "full":
            with ExitStack() as c2:
                ppd = c2.enter_context(tc.tile_pool(name="projd", bufs=4))
                ps4 = c2.enter_context(tc.tile_pool(name="projps", bufs=1,
                                                    space="PSUM"))
                wfp = c2.enter_context(tc.tile_pool(name="wfc", bufs=1))
                # all of Wf resident: one stationary load serves all 8 vocab
                # chunks of a row tile (LDWEIGHTS amortized 8x)
                wf_all = wfp.tile([128, 8, VSH], bf)
                for kk in range(8):
                    nc.scalar.dma_start(out=wf_all[:, kk, :],
                                        in_=Wfs[128 * kk:128 * (kk + 1), :])
                bf_all = wfp.tile([128, VSH], f32)
                nc.scalar.dma_start(out=bf_all[:],
                                    in_=bfs[:].to_broadcast([128, VSH]))
                pj_t = [ps4.tile([128, 500], f32, name=f"pj{i}")
                        for i in range(8)]
                for r in range(NC):
                    for th in range(2):
                        t0 = 32 * th
                        t1 = min(t0 + 32, TD)
                        rr = (t1 - t0) * NB
                        r0 = 252 * r + NB * t0
                        for kk in range(8):
                            lh = sb_ag[:, r, kk, t0:t1, :].rearrange(
                                "p t b -> p (t b)")
                            for sc in range(8):
                                nc.tensor.matmul(
                                    pj_t[sc][:rr, :], lh,
                                    wf_all[:, kk, 500 * sc:500 * (sc + 1)],
                                    start=(kk == 0), stop=(kk == 7))
                        for sc in range(8):
                            st = ppd.tile([128, 500], f32, tag="st")
                            nc.vector.tensor_add(
                                st[:rr, :], pj_t[sc][:rr, :],
                                bf_all[:rr, 500 * sc:500 * (sc + 1)])
                            nc.sync.dma_start(
                                out=logits[r0:r0 + rr,
                                           500 * sc:500 * (sc + 1)],
                                in_=st[:rr, :])

        if stage != "full":
            # partial-stage dummy output so the NEFF has its ExternalOutput written
            st0 = gp.tile([1, 4], f32, tag="dummy")
            nc.vector.tensor_copy(st0[:], tga[0:1, 0:4])
            nc.sync.dma_start(out=logits[0:1, 0:4], in_=st0[:])

    nc.finalize()
    return nc, dbg


_CACHE = {}


def _get_nc(stage="full", debug=False):
    key = (stage, debug)
    if key not in _CACHE:
        _CACHE[key] = _build_nc(stage, debug)
    return _CACHE[key]


def run_cores(inputs, stage="full", debug=False, trace=False):
    from concourse.bass_utils import run_bass_kernel_spmd
    shared, per_core = _prep_host(inputs)
    nc, dbg = _get_nc(stage, debug)
    in_maps = []
    for k in range(NC):
        m = dict(shared)
        m.update(per_core[k])
        in_maps.append(m)
    return run_bass_kernel_spmd(nc, in_maps, core_ids=list(range(NC)), trace=trace)


def unshard(outs):
    full = np.concatenate(outs, axis=1)                     # [2016, 32000]
    # rows ordered (r, t, b_local); batch b = 4*r + b_local
    full = full.reshape(NC, TD, NB, VT).transpose(0, 2, 1, 3).reshape(B, TD, VT)
    return np.ascontiguousarray(full.astype(np.float32))


def kernel(**inputs):
    res = run_cores(inputs, stage="full")
    outs = [np.asarray(r["logits"]) for r in res.results]   # [2016, 4000] each
    return unshard(outs)


# revision 33
# speedup vs baseline: 1.4215x; 1.0505x over previous
"""Trainium2 Bass kernel for nn_DmTranslateTrain (seq2seq translate train step).

Strategy (8 NeuronCores, SPMD):
  - Data-parallel over batch: core k owns batches [4k, 4k+4). Each core runs the
    full encoder LSTM scan + decoder (LSTM + Luong attention) for its 4 batches.
  - Output projection is tensor-parallel over the vocabulary: chunked AllGather
    of attention activations overlapped with the decoder, then each core
    computes logits[:, 4000k:4000k+4000].

Scan-step design (the hot loop):
  - The x-projection (emb @ Wx + b, precomputed in DRAM) is folded into the PE
    accumulation with a tiny K=4 identity matmul, so the gate nonlinearities
    read PSUM directly (no vector adds on the critical path).
  - All four gates use plain tanh: sigma(x) = (1+tanh(x/2))/2, with the 0.5
    pre-scale for gates i/f/o folded into the weight columns host-side.  One
    fused tanh per 512-col gate pair (2 ACTs per step), one activation table.
  - State kept scaled: C2 = 2c, H = 2h.  Updates via scalar_tensor_tensor:
      IG2 = (ti+1)*tu; FC2 = (tf+1)*C2; C2' = 0.5*FC2 + IG2;
      tc = tanh(0.5*C2'); H = (to+1)*tc.
  - z matmuls emitted kk-outer / m-inner so the 4 PE column-groups
    (tile_position=(0,32m)) stream concurrently; decoder scores run in column
    group q96 on separate PSUM partitions.

Gate packing: z tile is [128, 1024] per band m (partition = 32*m + b), free
col = gate*256 + 32*fc + r for unit u = 128*fc + 32*m + r, gates ordered
[u, i, f, o] (u = candidate).  The DVE 32x32 block transpose of the H tile
directly yields H^T in natural u-major chunks (one copy per step).
Logits rows are ordered (core, t, local batch); the host unshards.
"""

import numpy as np

B, TS, TD = 32, 64, 63
VS, VT = 32000, 32000
E, U = 256, 1024
G4 = 4 * U
NB = 4            # batches per core
NC = 8            # cores
VSH = VT // NC    # vocab shard per core
RE = TS * NB      # encoder rows per core
RD = TD * NB      # decoder rows per core
RT = TD * B       # total decoder rows (all batches)

_GATE_PERM = [2, 0, 1, 3]  # new order [u, i, f, o] -> original gate index
CHUNKS = [(0, 16), (16, 32), (32, 48), (48, 60), (60, TD)]


def _reorder_cols(w):
    # natural col = gate_orig*1024 + u, u = 128*fc + 32*m + r
    w5 = w.reshape(w.shape[0], 4, 8, 4, 32)        # [in, g_orig, fc, m, r]
    w5 = w5[:, _GATE_PERM]                          # [in, g_new, fc, m, r]
    w5 = w5.transpose(0, 3, 1, 2, 4)                # [in, m, g_new, fc, r]
    return np.ascontiguousarray(w5.reshape(w.shape[0], G4))


def _reorder_bias(b):
    b5 = b.reshape(4, 8, 4, 32)[_GATE_PERM].transpose(2, 0, 1, 3)
    return np.ascontiguousarray(b5.reshape(1, G4))


def _prep_host(inputs):
    import ml_dtypes
    bf16 = ml_dtypes.bfloat16
    f32 = np.float32
    enc_in = np.asarray(inputs["encoder_input"])
    dec_in = np.asarray(inputs["decoder_input"])
    Wx_e = np.asarray(inputs["Wx_e"], f32)
    Wh_e = np.asarray(inputs["Wh_e"], f32)
    b_e = np.asarray(inputs["b_e"], f32)
    Wx_d = np.asarray(inputs["Wx_d"], f32)
    Wh_d = np.asarray(inputs["Wh_d"], f32)
    b_d = np.asarray(inputs["b_d"], f32)
    Wm = np.asarray(inputs["Wm"], f32)
    Wa = np.asarray(inputs["Wa"], f32)
    Wf = np.asarray(inputs["Wf"], f32)
    bfv = np.asarray(inputs["bf"], f32)

    Wxd_x = Wx_d[:E]
    Wxd_a = Wx_d[E:]
    Wa_h, Wa_c = Wa[:U], Wa[U:]

    # per-gate column scale on the NATURAL layout (i, f, g, o): tanh trick
    # needs 0.5*z for i/f/o; the candidate gate g keeps full scale.
    cs = np.concatenate([np.full(U, 0.5, f32), np.full(U, 0.5, f32),
                         np.ones(U, f32), np.full(U, 0.5, f32)])

    shared = {
        "Wxe": _reorder_cols(Wx_e * cs).astype(bf16),
        "Whe": _reorder_cols(0.5 * Wh_e * cs).astype(bf16),
        "Whcomb": _reorder_cols(0.5 * (Wh_d + Wa_h @ Wxd_a) * cs).astype(bf16),
        "Wca": _reorder_cols(0.5 * (Wa_c @ Wxd_a) * cs).astype(bf16),
        "Whd0": _reorder_cols(0.5 * Wh_d * cs).astype(bf16),
        "Wxdx": _reorder_cols(Wxd_x * cs).astype(bf16),
        "Wm": (0.25 * Wm).astype(bf16),
        "WaH": (0.5 * Wa_h).astype(bf16),
        "WaC": np.ascontiguousarray((0.5 * Wa_c).astype(bf16)),
        "be": _reorder_bias(b_e * cs),
        "bd": _reorder_bias(b_d * cs),
        "enc_emb": np.ascontiguousarray(np.asarray(inputs["enc_emb"], f32)),
        "dec_emb": np.ascontiguousarray(np.asarray(inputs["dec_emb"], f32)),
    }
    Wf_bf = Wf.astype(bf16)
    per_core = []
    for k in range(NC):
        eidx = enc_in[NB * k:NB * (k + 1)]
        didx = dec_in[NB * k:NB * (k + 1)]
        per_core.append({
            "enc_idx": np.ascontiguousarray(eidx.T.reshape(RE, 1).astype(np.int32)),
            "dec_idx": np.ascontiguousarray(didx.T.reshape(RD, 1).astype(np.int32)),
            "Wfs": np.ascontiguousarray(Wf_bf[:, VSH * k:VSH * (k + 1)]),
            "bfs": np.ascontiguousarray(bfv[VSH * k:VSH * (k + 1)].reshape(1, VSH)),
        })
    return shared, per_core


# ---------------------------------------------------------------------------

def _build_nc(stage="full", debug=False):
    import re as _re
    from contextlib import ExitStack
    import concourse.bass as bass
    import concourse.mybir as mybir
    import concourse.tile as tile
    from concourse import bacc
    from concourse.masks import make_identity

    dt = mybir.dt
    AF = mybir.ActivationFunctionType
    ALU = mybir.AluOpType
    AX = mybir.AxisListType
    f32, bf = dt.float32, dt.bfloat16

    nc = bacc.Bacc("TRN2", target_bir_lowering=False, debug=False, num_devices=NC)

    enc_idx = nc.dram_tensor("enc_idx", [RE, 1], dt.int32, kind="ExternalInput")
    dec_idx = nc.dram_tensor("dec_idx", [RD, 1], dt.int32, kind="ExternalInput")
    enc_emb = nc.dram_tensor("enc_emb", [VS, E], f32, kind="ExternalInput")
    dec_emb = nc.dram_tensor("dec_emb", [VT, E], f32, kind="ExternalInput")
    Wxe = nc.dram_tensor("Wxe", [E, G4], bf, kind="ExternalInput")
    Whe = nc.dram_tensor("Whe", [U, G4], bf, kind="ExternalInput")
    Whcomb = nc.dram_tensor("Whcomb", [U, G4], bf, kind="ExternalInput")
    Wca_t = nc.dram_tensor("Wca", [U, G4], bf, kind="ExternalInput")
    Whd0 = nc.dram_tensor("Whd0", [U, G4], bf, kind="ExternalInput")
    Wxdx = nc.dram_tensor("Wxdx", [E, G4], bf, kind="ExternalInput")
    Wm_t = nc.dram_tensor("Wm", [U, U], bf, kind="ExternalInput")
    WaH_t = nc.dram_tensor("WaH", [U, U], bf, kind="ExternalInput")
    WaC_t = nc.dram_tensor("WaC", [U, U], bf, kind="ExternalInput")
    Wfs = nc.dram_tensor("Wfs", [U, VSH], bf, kind="ExternalInput")
    bfs = nc.dram_tensor("bfs", [1, VSH], f32, kind="ExternalInput")
    be_t = nc.dram_tensor("be", [1, G4], f32, kind="ExternalInput")
    bd_t = nc.dram_tensor("bd", [1, G4], f32, kind="ExternalInput")

    logits = nc.dram_tensor("logits", [RT, VSH], f32, kind="ExternalOutput")

    dbg = {}
    if debug:
        dbg["memT"] = nc.dram_tensor("dbg_memT", [128, 8, TS, NB], bf, kind="ExternalOutput")
        dbg["c_enc"] = nc.dram_tensor("dbg_cenc", [128, 256], f32, kind="ExternalOutput")
        dbg["keysT"] = nc.dram_tensor("dbg_keysT", [128, 8, NB, TS], bf, kind="ExternalOutput")
        dbg["HallT"] = nc.dram_tensor("dbg_HallT", [128, 8, TD + 1, NB], bf, kind="ExternalOutput")
        dbg["alTall"] = nc.dram_tensor("dbg_alTall", [128, 2, TD, NB], bf, kind="ExternalOutput")
        dbg["MemWca"] = nc.dram_tensor("dbg_MemWca", [128, 2, G4], bf, kind="ExternalOutput")

    with tile.TileContext(nc) as tc, ExitStack() as ctx:
        constp = ctx.enter_context(tc.tile_pool(name="const", bufs=1))
        ident = constp.tile([128, 128], bf)
        make_identity(nc, ident[:])

        dramp = ctx.enter_context(tc.tile_pool(name="dram", bufs=1, space="DRAM"))
        Xe_d = dramp.tile([RE, G4], bf, tag="Xe")
        Xd_d = dramp.tile([RD, G4], bf, tag="Xd")
        aginC = [dramp.tile([8, 128, (c1 - c0) * NB], bf, tag=f"agin{j}",
                            name=f"aginC{j}")
                 for j, (c0, c1) in enumerate(CHUNKS)]
        agoutC = [dramp.tile([NC, 8, 128, (c1 - c0) * NB], bf, tag=f"agout{j}",
                             name=f"agoutC{j}", addr_space="Shared")
                  for j, (c0, c1) in enumerate(CHUNKS)]

        statep = ctx.enter_context(tc.tile_pool(name="state", bufs=1))
        memT = statep.tile([128, 8, TS, NB], bf)       # encoder H^T (= 2h)
        C2 = statep.tile([128, 256], f32)              # 2c (enc then dec)
        keysT = statep.tile([128, 8, NB, TS], bf)      # keys^T, batch-major
        HdecT = statep.tile([128, 8, TD + 1, NB], bf)  # slot t+1 = H_t = 2h_t
        alTall = statep.tile([128, 2, TD, NB], bf)     # block-diag align rows=(q,s), cols=b
        MemWca = statep.tile([128, 2, G4], bf)         # (memT @ Wca'), rows=(q,s)
        MemWaC = statep.tile([128, 2, U], bf)          # (memT @ WaC'), rows=(q,s)

        gp = ctx.enter_context(tc.tile_pool(name="gates", bufs=1))
        xe_pp = [gp.tile([NB, G4], bf, name=f"xe{i}") for i in range(2)]
        tga = gp.tile([128, 512], f32)   # tanh(z_u), tanh(z_i/2)
        tfo = gp.tile([128, 512], f32)   # tanh(z_f/2), tanh(z_o/2)
        IG2 = gp.tile([128, 256], f32)
        FC2 = gp.tile([128, 256], f32)
        tc_t = gp.tile([128, 256], f32)
        Hbf = gp.tile([128, 256], bf)
        h_tr = gp.tile([128, 256], bf, tag="h_tr")

        # ------------- embedding gathers + X precomputes -------------
        def x_precompute_all(jobs):
            with ExitStack() as c2:
                pp = c2.enter_context(tc.tile_pool(name="xpre", bufs=2))
                pp1 = c2.enter_context(tc.tile_pool(name="xpre1", bufs=1))
                psx = c2.enter_context(tc.tile_pool(name="xpre_ps", bufs=1, space="PSUM"))
                tiles = []
                for jj, (idx_t, emb_t, w_t, bias_t, rows, out_d) in enumerate(jobs):
                    nm = (rows + 127) // 128
                    for m in range(nm):
                        r0 = 128 * m
                        rr = min(128 * (m + 1), rows) - r0
                        idx_sb = pp1.tile([128, 1], dt.int32, name=f"idx{jj}_{m}")
                        nc.sync.dma_start(out=idx_sb[:rr, :], in_=idx_t[r0:r0 + rr, :])
                        gath = pp1.tile([128, E], f32, name=f"gath{jj}_{m}")
                        nc.gpsimd.indirect_dma_start(
                            out=gath[:rr, :], out_offset=None,
                            in_=emb_t[:],
                            in_offset=bass.IndirectOffsetOnAxis(ap=idx_sb[:rr, :1],
                                                                axis=0))
                        gbf = pp1.tile([128, E], bf, name=f"gbf{jj}_{m}")
                        nc.vector.tensor_copy(gbf[:rr, :], gath[:rr, :])
                        tiles.append((jj, r0, rr, gbf))
                w_sb = pp1.tile([128, 2, G4], bf, name="wx")
                bias_bc = pp1.tile([128, G4], f32, name="biasbc")
                cur = [None]

                def _stage_wb(jj):
                    w_t, bias_t = jobs[jj][2], jobs[jj][3]
                    for kk in range(2):
                        nc.scalar.dma_start(out=w_sb[:, kk, :],
                                            in_=w_t[128 * kk:128 * (kk + 1), :])
                    nc.scalar.dma_start(out=bias_bc[:],
                                        in_=bias_t[:].to_broadcast([128, G4]))
                    cur[0] = jj

                for jj, r0, rr, gbf in tiles:
                    if cur[0] != jj:
                        _stage_wb(jj)
                    out_d = jobs[jj][5]
                    xT = pp.tile([128, 2, 128], bf, tag="xT")
                    for kk in range(2):
                        pt = psx.tile([128, 128], bf, tag="ptr")
                        nc.tensor.transpose(pt[:, :rr], gbf[:rr, 128 * kk:128 * (kk + 1)],
                                            ident[:rr, :rr])
                        nc.vector.tensor_copy(xT[:, kk, :rr], pt[:, :rr])
                    for chv in range(8):
                        cs0 = 512 * chv
                        ps = psx.tile([128, 512], f32, tag="pmm")
                        for kk in range(2):
                            nc.tensor.matmul(ps[:rr, :], xT[:, kk, :rr],
                                             w_sb[:, kk, cs0:cs0 + 512],
                                             start=(kk == 0), stop=(kk == 1))
                        st = pp.tile([128, 512], bf, tag="stage")
                        nc.vector.tensor_add(st[:rr, :], ps[:rr, :],
                                             bias_bc[:rr, cs0:cs0 + 512])
                        nc.sync.dma_start(out=out_d[r0:r0 + rr, cs0:cs0 + 512],
                                          in_=st[:rr, :])

        def gate_tail(ps, dst_of_h):
            # z in psum ps [128, 1024]; writes H^T into dst_of_h(h) [128, 4, NB]
            # for kk half h, updates C2 in place.  Split into fc-halves so the
            # first half of H^T (kk 0..3) lands early and the next z-stream
            # restarts sooner.
            ps4 = ps[:].rearrange("p (g c) -> p g c", g=4)
            tga4 = tga[:].rearrange("p (g c) -> p g c", g=2)
            tfo4 = tfo[:].rearrange("p (g c) -> p g c", g=2)
            for h in range(2):
                cl, ch = 128 * h, 128 * h + 128
                nc.scalar.activation(tga4[:, :, cl:ch], ps4[:, 0:2, cl:ch],
                                     AF.Tanh)
                nc.scalar.activation(tfo4[:, :, cl:ch], ps4[:, 2:4, cl:ch],
                                     AF.Tanh)
                nc.vector.scalar_tensor_tensor(
                    IG2[:, cl:ch], tga[:, 256 + cl:256 + ch], 1.0,
                    tga[:, cl:ch], op0=ALU.add, op1=ALU.mult)
                nc.vector.scalar_tensor_tensor(
                    FC2[:, cl:ch], tfo[:, cl:ch], 1.0,
                    C2[:, cl:ch], op0=ALU.add, op1=ALU.mult)
                nc.vector.scalar_tensor_tensor(
                    C2[:, cl:ch], FC2[:, cl:ch], 0.5,
                    IG2[:, cl:ch], op0=ALU.mult, op1=ALU.add)
                nc.scalar.activation(tc_t[:, cl:ch], C2[:, cl:ch],
                                     AF.Tanh, scale=0.5)
                nc.vector.scalar_tensor_tensor(
                    Hbf[:, cl:ch], tfo[:, 256 + cl:256 + ch], 1.0,
                    tc_t[:, cl:ch], op0=ALU.add, op1=ALU.mult)
                nc.vector.transpose(h_tr[:, cl:ch], Hbf[:, cl:ch])
                nc.vector.tensor_copy(
                    dst_of_h(h),
                    h_tr[:, cl:ch].rearrange("p (k c) -> p k c", k=4)[:, :, 0:NB])

        # gathered attention activations: scattered per AllGather chunk during
        # the decoder, consumed by the projection after the scan scope closes.
        # Must sit below the scan pools in the pool stack.
        sbagp = ctx.enter_context(tc.tile_pool(name="sbag", bufs=1))
        sb_ag = sbagp.tile([128, NC, 8, TD, NB], bf)

        # ------------- scans (shared psum pool) -------------
        with ExitStack() as scn:
            psp = scn.enter_context(tc.tile_pool(name="scanps", bufs=1, space="PSUM"))
            psum_z0 = psp.tile([128, 1024], f32, tag="pz0")
            psum_z1 = psp.tile([128, 1024], f32, tag="pz1")
            psum_zp = [psum_z0, psum_z1]
            psum_sc = psp.tile([128, 256], f32, tag="psc")
            psum_mw = psp.tile([128, 512], f32, tag="pmw")

            def emit_ids(ps, xe, close):
                # identity matmuls fold the x projection into psum (group start)
                for m in range(4):
                    for chv in range(2):
                        co = 1024 * m + 512 * chv
                        nc.tensor.matmul(
                            ps[32 * m:32 * m + NB, 512 * chv:512 * chv + 512],
                            ident[0:NB, 0:NB], xe[0:NB, co:co + 512],
                            start=True, stop=close,
                            tile_position=(0, 32 * m))

            def emit_z_stream(ps, lhsT_of_kk, w_sb_of_kk, with_align,
                              al_t=None):
                # kk-outer, chv-inner: per col group the two chv matmuls share
                # one stationary load (bass skips the redundant LDWEIGHTS)
                for kk in range(8):
                    lh = lhsT_of_kk(kk)
                    for m in range(4):
                        for chv in range(2):
                            co = 1024 * m + 512 * chv
                            nc.tensor.matmul(
                                ps[32 * m:32 * m + NB, 512 * chv:512 * chv + 512],
                                lh, w_sb_of_kk(kk)[:, co:co + 512],
                                start=False,
                                stop=(kk == 7 and not with_align),
                                tile_position=(0, 32 * m))
                if with_align:
                    for m in range(4):
                        for p in range(2):
                            for chv in range(2):
                                co = 1024 * m + 512 * chv
                                nc.tensor.matmul(
                                    ps[32 * m:32 * m + NB, 512 * chv:512 * chv + 512],
                                    alTall[:, p, al_t, :],
                                    MemWca[:, p, co:co + 512],
                                    start=False, stop=(p == 1),
                                    tile_position=(0, 32 * m))

            # x precompute first: its staging pools need the space the big
            # weight pools occupy later.
            x_precompute_all([
                (enc_idx, enc_emb, Wxe, be_t, RE, Xe_d),
                (dec_idx, dec_emb, Wxdx, bd_t, RD, Xd_d),
            ])

            # Whcomb: 6 chunks prefetched during the encoder (gpsimd queue
            # is idle); the last 2 chunks load once Wca's space frees up.
            whcp = scn.enter_context(tc.tile_pool(name="whc", bufs=1))
            whc_a = whcp.tile([128, 6, G4], bf)
            whc_b = None

            def whc_of_kk(kk):
                return whc_a[:, kk, :] if kk < 6 else whc_b[:, kk - 6, :]

            # ---------------- encoder ----------------
            with ExitStack() as ec:
                encp = ec.enter_context(tc.tile_pool(name="enc", bufs=1))
                whe_sb = encp.tile([128, 8, G4], bf)
                for kk in range(8):
                    nc.scalar.dma_start(out=whe_sb[:, kk, :],
                                        in_=Whe[128 * kk:128 * (kk + 1), :])

                nc.vector.memset(C2[:], 0.0)

                nc.sync.dma_start(out=xe_pp[0][:], in_=Xe_d[0:NB, :])
                emit_ids(psum_zp[0], xe_pp[0], close=True)
                for t in range(TS):
                    ps = psum_zp[t % 2]
                    if t + 1 < TS:
                        nc.sync.dma_start(out=xe_pp[(t + 1) % 2][:],
                                          in_=Xe_d[NB * (t + 1):NB * (t + 2), :])
                    if t > 0:
                        emit_z_stream(ps,
                                      (lambda kk, _t=t: memT[:, kk, _t - 1, :]),
                                      (lambda kk: whe_sb[:, kk, :]),
                                      with_align=False)
                    # next step's id matmuls go in front of the tail so they
                    # fill the PE gap (they only need the x tile)
                    if t + 1 < TS:
                        emit_ids(psum_zp[(t + 1) % 2], xe_pp[(t + 1) % 2],
                                 close=False)
                    gate_tail(ps, (lambda h, _t=t:
                                   memT[:, 4 * h:4 * h + 4, _t, :]))
                    # prefetch most of Whcomb on the idle gpsimd queue
                    # (last 2 chunks wait for the Wca space at the transition)
                    if stage != "enc" and t == 40:
                        for kk in range(6):
                            nc.gpsimd.dma_start(
                                out=whc_a[:, kk, :],
                                in_=Whcomb[128 * kk:128 * (kk + 1), :])

                if debug:
                    nc.sync.dma_start(out=dbg["memT"][:], in_=memT[:])
                    nc.sync.dma_start(out=dbg["c_enc"][:], in_=C2[:])

            # ---------------- transition: keys, MemWca, MemWaC ----------------
            m_dec = _re.match(r"dec(\d+)$", stage)
            TD_RUN = int(m_dec.group(1)) if m_dec else TD
            if stage != "enc":
                decp = scn.enter_context(tc.tile_pool(name="dec", bufs=1))

                memQ = decp.tile([128, 8, 2, 128], bf)

                with ExitStack() as c3:
                    wmp = c3.enter_context(tc.tile_pool(name="wmp", bufs=1))
                    wm_sb = wmp.tile([128, 8, U], bf)
                    for kk in range(8):
                        nc.gpsimd.dma_start(out=wm_sb[:, kk, :],
                                            in_=Wm_t[128 * kk:128 * (kk + 1), :])
                    # keysT = (memT @ Wm')^T, stored batch-major [p, kk, b, s]
                    for ko in range(8):
                        for kk in range(8):
                            nc.tensor.matmul(psum_mw[:, 0:256],
                                             wm_sb[:, kk, 128 * ko:128 * (ko + 1)],
                                             memT[:, kk, :, :],
                                             start=(kk == 0), stop=(kk == 7))
                        nc.vector.tensor_copy(
                            keysT[:, ko],
                            psum_mw[:, 0:256].rearrange("p (s b) -> p b s", b=NB))

                    # memQ[:, kk, p, 64q+s] = memT[:, kk, s, 2p+q]
                    for kk in range(8):
                        for p in range(2):
                            nc.vector.tensor_copy(
                                memQ[:, kk, p, :].rearrange("p (q s) -> p q s", q=2),
                                memT[:, kk, :, 2 * p:2 * p + 2].rearrange(
                                    "p s q -> p q s"))

                with ExitStack() as c3b:
                    wcap2 = c3b.enter_context(tc.tile_pool(name="wca2", bufs=1))
                    wca_sb = wcap2.tile([128, 8, G4], bf)
                    for kk in range(8):
                        nc.gpsimd.dma_start(out=wca_sb[:, kk, :],
                                            in_=Wca_t[128 * kk:128 * (kk + 1), :])
                    for p in range(2):
                        for c8 in range(8):
                            for kk in range(8):
                                nc.tensor.matmul(
                                    psum_mw[:], memQ[:, kk, p, :],
                                    wca_sb[:, kk, 512 * c8:512 * (c8 + 1)],
                                    start=(kk == 0), stop=(kk == 7))
                            nc.vector.tensor_copy(
                                MemWca[:, p, 512 * c8:512 * (c8 + 1)], psum_mw[:])

                whcp2 = scn.enter_context(tc.tile_pool(name="whc2", bufs=1))
                whc_b = whcp2.tile([128, 2, G4], bf)
                for kk in range(6, 8):
                    nc.gpsimd.dma_start(out=whc_b[:, kk - 6, :],
                                        in_=Whcomb[128 * kk:128 * (kk + 1), :])
                # decoder x tiles can load as soon as the encoder stops
                # touching the ping-pong buffers
                nc.sync.dma_start(out=xe_pp[0][:], in_=Xd_d[0:NB, :])
                nc.sync.dma_start(out=xe_pp[1][:], in_=Xd_d[NB:2 * NB, :])

                with ExitStack() as c3c:
                    wacp = c3c.enter_context(tc.tile_pool(name="wacp", bufs=1))
                    wac_sb = wacp.tile([128, 8, U], bf)
                    for kk in range(8):
                        nc.gpsimd.dma_start(out=wac_sb[:, kk, :],
                                            in_=WaC_t[128 * kk:128 * (kk + 1), :])
                    for p in range(2):
                        for c2_ in range(2):
                            for kk in range(8):
                                nc.tensor.matmul(
                                    psum_mw[:], memQ[:, kk, p, :],
                                    wac_sb[:, kk, 512 * c2_:512 * (c2_ + 1)],
                                    start=(kk == 0), stop=(kk == 7))
                            nc.vector.tensor_copy(
                                MemWaC[:, p, 512 * c2_:512 * (c2_ + 1)], psum_mw[:])

                if debug:
                    nc.sync.dma_start(out=dbg["keysT"][:], in_=keysT[:])
                    nc.sync.dma_start(out=dbg["MemWca"][:], in_=MemWca[:])

                # ---------------- decoder scan ----------------
                nc.vector.memset(alTall[:], 0.0)

                rsums = decp.tile([128, NB], f32)
                rmask = decp.tile([128, NB], f32)
                rsD = decp.tile([128, 1], f32)
                # rmask[96+p, b] = 1 iff p == b (diag selector)
                nc.vector.tensor_copy(rmask[96:128, :], ident[96:128, 96:96 + NB])

                exp_sc = None
                align_bf = None
                dve_t = None
                attnT = None
                wah_sb = None

                def softmax_emit(t):
                    # scores in psum_sc rows 96:100 -> alTall[:, :, t, :]
                    nc.scalar.activation(exp_sc[96:128, :], psum_sc[96:128, :], AF.Exp)
                    for b in range(NB):
                        nc.vector.reduce_sum(rsums[96:128, b:b + 1],
                                             exp_sc[96:128, 64 * b:64 * (b + 1)],
                                             axis=AX.X)
                    nc.vector.tensor_mul(rsums[96:128, :], rsums[96:128, :],
                                         rmask[96:128, :])
                    nc.vector.reduce_sum(rsD[96:128, :], rsums[96:128, :], axis=AX.X)
                    nc.vector.reciprocal(rsD[96:128, :], rsD[96:128, :])
                    nc.vector.tensor_scalar(align_bf[96:128, :], exp_sc[96:128, :],
                                            rsD[96:128, 0:1], None, op0=ALU.mult)
                    nc.vector.transpose(dve_t[96:128, :], align_bf[96:128, :])
                    # diag value align_b[32h+r] sits at dve_t[96+r, 32*(2b+h)+b]
                    for b in range(NB):
                        p, q = b // 2, b % 2
                        for hh in range(2):
                            cc = 32 * (2 * b + hh) + b
                            nc.vector.tensor_copy(
                                alTall[64 * q + 32 * hh:64 * q + 32 * hh + 32,
                                       p, t, b:b + 1],
                                dve_t[96:128, cc:cc + 1])

                def attn_chunk(j):
                    c0, c1 = CHUNKS[j]
                    cw = (c1 - c0) * NB
                    for ko in range(8):
                        pa = psum_mw[:, 0:cw]
                        for kk in range(8):
                            nc.tensor.matmul(
                                pa, wah_sb[:, kk, 128 * ko:128 * (ko + 1)],
                                HdecT[:, kk, 1 + c0:1 + c1, :],
                                start=(kk == 0), stop=False)
                        for p in range(2):
                            nc.tensor.matmul(
                                pa,
                                MemWaC[:, p, 128 * ko:128 * (ko + 1)],
                                alTall[:, p, c0:c1, :].rearrange(
                                    "p t b -> p (t b)"),
                                start=False, stop=(p == 1))
                        nc.vector.tensor_copy(attnT[:, ko, 0:cw], pa)
                    nc.gpsimd.dma_start(
                        out=aginC[j][:].rearrange("k p c -> p k c"),
                        in_=attnT[:, :, 0:cw])
                    nc.gpsimd.collective_compute(
                        "AllGather", ALU.bypass,
                        ins=[aginC[j][:]], outs=[agoutC[j][:]],
                        replica_groups=[list(range(NC))])
                    for r in range(NC):
                        nc.gpsimd.dma_start(
                            out=sb_ag[:, r, :, c0:c1, :],
                            in_=agoutC[j][r].rearrange("k p (t b) -> p k t b",
                                                       b=NB))

                def scores_emit(t):
                    for kk in range(8):
                        nc.tensor.matmul(
                            psum_sc[96:96 + NB, :],
                            HdecT[:, kk, t + 1, :],
                            keysT[:, kk].rearrange("p b s -> p (b s)"),
                            start=(kk == 0), stop=(kk == 7),
                            tile_position=(0, 96))

                # streamed t=0 weights (Whd0) in a scoped pool
                with ExitStack() as c4:
                    w0p = c4.enter_context(tc.tile_pool(name="w0", bufs=3))
                    w0_tiles = []
                    for kk in range(8):
                        w0 = w0p.tile([128, G4], bf, tag="w0")
                        nc.gpsimd.dma_start(out=w0[:],
                                            in_=Whd0[128 * kk:128 * (kk + 1), :])
                        w0_tiles.append(w0)

                    ps = psum_zp[0]
                    emit_ids(ps, xe_pp[0], close=False)
                    emit_z_stream(ps,
                                  (lambda kk: memT[:, kk, TS - 1, :]),
                                  (lambda kk: w0_tiles[kk]),
                                  with_align=False)
                    emit_ids(psum_zp[1], xe_pp[1], close=False)
                    gate_tail(ps, (lambda h: HdecT[:, 4 * h:4 * h + 4, 1, :]))
                    scores_emit(0)

                # softmax scratch + attn staging + WaH + gathered activations
                # (allocated after the w0 pool frees its space)
                dec2p = scn.enter_context(tc.tile_pool(name="dec2", bufs=1))
                exp_sc = dec2p.tile([128, 256], f32)
                align_bf = dec2p.tile([128, 256], bf)
                dve_t = dec2p.tile([128, 256], bf)
                attnT = dec2p.tile([128, 8, 64], bf)   # per-chunk staging
                wah_sb = dec2p.tile([128, 8, U], bf)
                # wah rides the idle gpsimd queue; needed first at t=15
                for kk in range(8):
                    nc.gpsimd.dma_start(out=wah_sb[:, kk, :],
                                        in_=WaH_t[128 * kk:128 * (kk + 1), :])
                softmax_emit(0)

                for t in range(1, TD_RUN):
                    ps = psum_zp[t % 2]
                    if t + 1 < TD_RUN:
                        nc.sync.dma_start(out=xe_pp[(t + 1) % 2][:],
                                          in_=Xd_d[NB * (t + 1):NB * (t + 2), :])
                    emit_z_stream(ps,
                                  (lambda kk, _t=t: HdecT[:, kk, _t, :]),
                                  whc_of_kk,
                                  with_align=True, al_t=t - 1)
                    if t + 1 < TD_RUN:
                        emit_ids(psum_zp[(t + 1) % 2], xe_pp[(t + 1) % 2],
                                 close=False)
                    gate_tail(ps, (lambda h, _t=t:
                                   HdecT[:, 4 * h:4 * h + 4, _t + 1, :]))
                    scores_emit(t)
                    softmax_emit(t)
                    if stage == "full" and (t + 1) in [c1 for _, c1 in CHUNKS]:
                        attn_chunk([c1 for _, c1 in CHUNKS].index(t + 1))

                if debug:
                    nc.sync.dma_start(out=dbg["HallT"][:], in_=HdecT[:])
                    nc.sync.dma_start(out=dbg["alTall"][:], in_=alTall[:])

        # ------- projection (sb_ag filled by the chunked AllGather) -------
        if stage == "full":
            with ExitStack() as c2:
                ppd = c2.enter_context(tc.tile_pool(name="projd", bufs=4))
                ps4 = c2.enter_context(tc.tile_pool(name="projps", bufs=1,
                                                    space="PSUM"))
                wfp = c2.enter_context(tc.tile_pool(name="wfc", bufs=1))
                # all of Wf resident: one stationary load serves all 8 vocab
                # chunks of a row tile (LDWEIGHTS amortized 8x)
                wf_all = wfp.tile([128, 8, VSH], bf)
                for kk in range(8):
                    nc.scalar.dma_start(out=wf_all[:, kk, :],
                                        in_=Wfs[128 * kk:128 * (kk + 1), :])
                bf_all = wfp.tile([128, VSH], f32)
                nc.scalar.dma_start(out=bf_all[:],
                                    in_=bfs[:].to_broadcast([128, VSH]))
                pj_t = [ps4.tile([128, 500], f32, name=f"pj{i}")
                        for i in range(8)]
                for r in range(NC):
                    for th in range(2):
                        t0 = 32 * th
                        t1 = min(t0 + 32, TD)
                        rr = (t1 - t0) * NB
                        r0 = 252 * r + NB * t0
                        lhs = [sb_ag[:, r, kk, t0:t1, :].rearrange(
                                   "p t b -> p (t b)") for kk in range(8)]
                        # sc-outer: each vocab chunk's accumulation group
                        # closes early so its bias-add and output DMA overlap
                        # the next chunk's matmuls
                        for sc in range(8):
                            for kk in range(8):
                                nc.tensor.matmul(
                                    pj_t[sc][:rr, :], lhs[kk],
                                    wf_all[:, kk, 500 * sc:500 * (sc + 1)],
                                    start=(kk == 0), stop=(kk == 7))
                            st = ppd.tile([128, 500], f32, tag="st")
                            nc.vector.tensor_add(
                                st[:rr, :], pj_t[sc][:rr, :],
                                bf_all[:rr, 500 * sc:500 * (sc + 1)])
                            nc.sync.dma_start(
                                out=logits[r0:r0 + rr,
                                           500 * sc:500 * (sc + 1)],
                                in_=st[:rr, :])

        if stage != "full":
            # partial-stage dummy output so the NEFF has its ExternalOutput written
            st0 = gp.tile([1, 4], f32, tag="dummy")
            nc.vector.tensor_copy(st0[:], tga[0:1, 0:4])
            nc.sync.dma_start(out=logits[0:1, 0:4], in_=st0[:])

    nc.finalize()
    return nc, dbg


_CACHE = {}


def _get_nc(stage="full", debug=False):
    key = (stage, debug)
    if key not in _CACHE:
        _CACHE[key] = _build_nc(stage, debug)
    return _CACHE[key]


def run_cores(inputs, stage="full", debug=False, trace=False):
    from concourse.bass_utils import run_bass_kernel_spmd
    shared, per_core = _prep_host(inputs)
    nc, dbg = _get_nc(stage, debug)
    in_maps = []
    for k in range(NC):
        m = dict(shared)
        m.update(per_core[k])
        in_maps.append(m)
    return run_bass_kernel_spmd(nc, in_maps, core_ids=list(range(NC)), trace=trace)


def unshard(outs):
    full = np.concatenate(outs, axis=1)                     # [2016, 32000]
    # rows ordered (r, t, b_local); batch b = 4*r + b_local
    full = full.reshape(NC, TD, NB, VT).transpose(0, 2, 1, 3).reshape(B, TD, VT)
    return np.ascontiguousarray(full.astype(np.float32))


def kernel(**inputs):
    res = run_cores(inputs, stage="full")
    outs = [np.asarray(r["logits"]) for r in res.results]   # [2016, 4000] each
    return unshard(outs)
